# revision 32
# baseline (speedup 1.0000x reference)
"""Trainium2 Bass kernel for a 2-layer GENConv (softmax aggr) + LayerNorm GNN block.

Distribution: graph-partitioned across 8 NeuronCores. Nodes are reordered by a
Fiedler-vector (spectral 1D) layout so the adjacency becomes banded; the
per-channel softmax aggregation collapses to two banded-SpMM matmuls because
GENConv softmax logits depend only on the source node:

  r = relu(x); w = exp(t*r); q = w*r
  num = A @ q;  den = A @ w;  agg = num/den        (the max-shift cancels; the
  1e-7 message eps shifts agg by exactly 1e-7 — far below tolerance — dropped)

Each core owns 4 contiguous dst blocks of 128 nodes; its banded A^T slab and
the qw window it contracts against are uniform across cores (SPMD), with
per-core variation expressed purely through input data (zero-padded bands).

Three SPMD launches (host work between them is pure data movement):
  A: conv1, software-pipelined (SpMM of block b+1 issued before the epilogue
     of block b so the PE never drains); emits x1 in bf16 and conv2's message
     tensors q2|w2 = relu(x1)*exp(t2*relu(x1)), exp(t2*relu(x1)).
  B: conv2 from the precomputed qw2 window (no window elementwise at all),
     LayerNorm via bn_stats, channel-major column sums off bf16 tiles
     (cs2 = cs_x1 + cs_relu(LN) — x2 itself is never materialized), and a
     per-core partial Wc matvec g_c = colsums_c @ Wc.T (bf16).
  C: tiny matmul-free finalize in channel-major [128,6] layout:
     row0 = sum_c g_c / n + bc + x0.
"""

import ml_dtypes
import numpy as np

import concourse.bass as bass
import concourse.bacc as bacc
import concourse.mybir as mybir
import concourse.tile as tile
import concourse.masks as masks
from concourse.bass_utils import run_bass_kernel_spmd

F32 = mybir.dt.float32
BF16 = mybir.dt.bfloat16
F8E4 = mybir.dt.float8e4
AF = mybir.ActivationFunctionType
ALU = mybir.AluOpType

N_CORES = 8
H = 768
CHT = H // 128           # channel tiles = 6
LN_EPS = 1e-5

_cache = {}


# ----------------------------------------------------------------------------
# Host-side graph preprocessing (index work only — no float math on data).
# ----------------------------------------------------------------------------

def _band_struct(ns, nd, n, bpc):
    """Per-block source-tile extents [st, en) (in 128-tiles) of the permuted
    adjacency, and the per-slot window ranges shared across cores."""
    nb = n // 128
    order = np.lexsort((ns, nd))
    ns_s, nd_s = ns[order], nd[order]
    starts = np.searchsorted(nd_s, np.arange(0, n, 128))
    ends = np.searchsorted(nd_s, np.arange(128, n + 1, 128))
    st = np.empty(nb, dtype=np.int64)
    en = np.empty(nb, dtype=np.int64)
    for b in range(nb):
        s = ns_s[starts[b]:ends[b]]
        st[b] = s.min() // 128
        en[b] = s.max() // 128 + 1
    pmax_t = int((np.arange(nb) - st).max())         # tiles left of own block
    # window origin (tile) for core c is c*bpc - pmax_t; slot ranges are the
    # max hull across cores in window-tile coordinates
    slot_lo, slot_hi = [], []
    for s in range(bpc):
        org = np.arange(N_CORES) * bpc - pmax_t
        blocks = np.arange(N_CORES) * bpc + s
        slot_lo.append(int((st[blocks] - org).min()))
        slot_hi.append(int((en[blocks] - org).max()))
    wlo = min(slot_lo)
    whi = max(slot_hi)
    slot_lo = [lo - wlo for lo in slot_lo]
    slot_hi = [hi - wlo for hi in slot_hi]
    return st, en, pmax_t, wlo, whi, slot_lo, slot_hi


def _ordering(src, dst, n, bpc):
    """1D layout minimizing the per-slot banded-SpMM tile count: scan linear
    directions in the span of Laplacian eigenvectors 1..3 (the near-degenerate
    first modes of a 3D point cloud mix arbitrarily; a pure axis mode gives
    the narrowest band). Falls back to the Fiedler vector / identity."""
    import scipy.sparse as sp
    a = sp.csr_matrix(
        (np.ones(len(src), dtype=np.float64), (dst, src)), shape=(n, n)
    )
    asym = ((a + a.T) > 0).astype(np.float64)
    try:
        from scipy.sparse.linalg import eigsh
        lap = sp.diags(np.asarray(asym.sum(1)).ravel()) - asym
        _, vecs = eigsh(lap, k=4, sigma=-1e-4, which="LM")
        emb = vecs[:, 1:4]
    except Exception:
        return np.arange(n, dtype=np.int64)

    inv = np.empty(n, dtype=np.int64)

    def cost(perm):
        inv[perm] = np.arange(n)
        _, _, _, wlo, whi, slot_lo, slot_hi = _band_struct(
            inv[src], inv[dst], n, bpc)
        return (sum(hi - lo for lo, hi in zip(slot_lo, slot_hi)), whi - wlo)

    rngs = np.random.RandomState(42)
    dirs = [np.eye(3)[i] for i in range(3)]
    dirs += [v / np.linalg.norm(v) for v in rngs.randn(240, 3)]
    best = None
    for u in dirs:
        perm = np.argsort(emb @ u).astype(np.int64)
        c = cost(perm)
        if best is None or c < best[0]:
            best = (c, perm)
    return best[1]


def _prepare(edge_index, n):
    import scipy.sparse as sp
    src = np.asarray(edge_index[0], dtype=np.int64)
    dst = np.asarray(edge_index[1], dtype=np.int64)
    nb = n // 128
    bpc = nb // N_CORES                     # blocks per core
    perm = _ordering(src, dst, n, bpc)      # new position i holds old node perm[i]
    inv = np.empty(n, dtype=np.int64)
    inv[perm] = np.arange(n)
    ns, nd = inv[src], inv[dst]             # edges in new coordinates

    st, en, pmax_t, wlo, whi, slot_lo, slot_hi = _band_struct(ns, nd, n, bpc)
    slot_S = [hi - lo for lo, hi in zip(slot_lo, slot_hi)]
    slot_off = np.concatenate([[0], np.cumsum(slot_S)]).astype(int)
    nxt = whi - wlo                          # window tiles per core
    own_off = pmax_t - wlo                   # window tile of first own block
    wx = nxt * 128

    # banded A^T slabs, packed per (slot, k) for contiguous DMA:
    # ab[c][p, (slot_off[s]+k)*128 + d] =
    #   #edges src=(win_org + (slot_lo[s]+k)*128 + p) -> dst=(blk(c,s)*128 + d)
    amat = sp.csr_matrix(
        (np.ones(len(ns), dtype=np.float64), (nd, ns)), shape=(n, n))
    abands = []
    for c in range(N_CORES):
        org = (c * bpc - pmax_t + wlo) * 128          # window row origin
        ab = np.zeros((128, slot_off[-1] * 128), dtype=np.float32)
        for s in range(bpc):
            blk = (c * bpc + s) * 128
            for k in range(slot_S[s]):
                r0 = org + (slot_lo[s] + k) * 128     # src rows of this tile
                a0, a1 = max(r0, 0), min(r0 + 128, n)
                if a0 >= a1:
                    continue
                sub = np.asarray(
                    amat[blk:blk + 128, a0:a1].todense(), dtype=np.float32)
                tilecol = (slot_off[s] + k) * 128
                ab[a0 - r0:a1 - r0, tilecol:tilecol + 128] = sub.T
        abands.append(ab.astype(ml_dtypes.float8_e4m3))

    return dict(perm=perm, inv=inv, pmax_t=pmax_t, wlo=wlo, nxt=nxt, wx=wx,
                own_off=own_off, slot_lo=slot_lo, slot_S=slot_S,
                slot_off=slot_off, bpc=bpc, abands=abands)


def _win_slice(full, prep, c):
    """Window rows of `full` for core c, zero-padded."""
    n = full.shape[0]
    bpc, pmax_t, wlo, wx = prep["bpc"], prep["pmax_t"], prep["wlo"], prep["wx"]
    lo = (c * bpc - pmax_t + wlo) * 128
    hi = lo + wx
    out = np.zeros((hi - lo, full.shape[1]), dtype=full.dtype)
    a, b = max(lo, 0), min(hi, n)
    out[a - lo:b - lo] = full[a:b]
    return out


# ----------------------------------------------------------------------------
# Shared Bass fragments.
# ----------------------------------------------------------------------------

def _spmm_block(nc, agg, ab_sb, qw, bl, prep):
    """agg[128,2H] (PSUM) += banded A^T slab tiles x qw window tiles.
    fp8 DoubleRow: each matmul contracts a PAIR of 128-row k-tiles
    (lhsT [128,2,128], rhs [128,2,512]) at 2x PE rate; odd tail single."""
    S = prep["slot_S"][bl]
    off = prep["slot_off"][bl]
    lo = prep["slot_lo"][bl]
    k = 0
    while k < S:
        pair = 2 if k + 1 < S else 1
        s = lo + k                      # first window tile of this pair
        last = k + pair >= S
        if pair == 2:
            at = ab_sb[:, (off + k) * 128:(off + k + 2) * 128].rearrange(
                "p (j d) -> p j d", d=128)
            qp = qw[:, s * 2 * H:(s + 2) * 2 * H].rearrange(
                "p (j d) -> p j d", d=2 * H)
            for ch in range(3):         # 1536 free = 3 x 512
                qv = qp[:, :, ch * 512:(ch + 1) * 512]
                nc.tensor.matmul(
                    agg[:, ch * 512:(ch + 1) * 512], at, qv,
                    start=(k == 0), stop=last,
                    perf_mode=mybir.MatmulPerfMode.DoubleRow,
                )
        else:
            at = ab_sb[:, (off + k) * 128:(off + k + 1) * 128]
            for ch in range(3):
                nc.tensor.matmul(
                    agg[:, ch * 512:(ch + 1) * 512], at,
                    qw[:, s * 2 * H + ch * 512:s * 2 * H + (ch + 1) * 512],
                    start=(k == 0), stop=last,
                )
        k += pair


def _div_res(nc, ep, agg, xo):
    """m_bf (bf16) = 16 * agg[:, :H] / agg[:, H:] + xo  (softmax divide +
    residual; the 16 undoes the q/16 fp8-range scaling of the messages).
    Runs in two half-width passes so the first transposes can start ~1.5us
    earlier (shorter critical chain on the last block)."""
    rec = ep.tile([128, H], F32, tag="rec")
    mtmp = ep.tile([128, H], F32, tag="mtmp")
    m_bf = ep.tile([128, H], BF16, tag="m_bf")
    hh = H // 2
    for i in range(2):
        sl = slice(i * hh, (i + 1) * hh)
        nc.vector.reciprocal_approx_fast(rec[:, sl], agg[:, H + i * hh:H + (i + 1) * hh])
        nc.vector.tensor_mul(mtmp[:, sl], agg[:, sl], rec[:, sl])
        nc.vector.scalar_tensor_tensor(m_bf[:, sl], mtmp[:, sl], 16.0, xo[:, sl],
                                       ALU.mult, ALU.add)
    return m_bf


def _mlp_block(nc, pW, ep, m_bf, wt_sb, ident, br_sb, xn):
    """xn[128,H] (SBUF f32) = m_bf @ W.T + b via 6 transposes + 12 matmuls.
    pW is a single-bank PSUM pool reused for the transposes and both
    output passes (sequential requests serialize safely)."""
    tp = pW.tile([128, H], BF16, tag="pw")
    for c in range(CHT):
        nc.tensor.transpose(tp[:, c * 128:(c + 1) * 128],
                            m_bf[:, c * 128:(c + 1) * 128], ident[:])
    mt = ep.tile([128, H], BF16, tag="mt")
    for c in range(CHT):
        nc.scalar.copy(mt[:, c * 128:(c + 1) * 128], tp[:, c * 128:(c + 1) * 128])
    xps1 = pW.tile([128, 512], F32, tag="pw")
    for c in range(CHT):
        nc.tensor.matmul(xps1[:], mt[:, c * 128:(c + 1) * 128],
                         wt_sb[:, c * H:c * H + 512],
                         start=(c == 0), stop=(c == CHT - 1))
    nc.vector.tensor_add(xn[:, 0:512], xps1[:], br_sb[:, 0:512])
    xps2 = pW.tile([128, 256], F32, tag="pw")
    for c in range(CHT):
        nc.tensor.matmul(xps2[:], mt[:, c * 128:(c + 1) * 128],
                         wt_sb[:, c * H + 512:(c + 1) * H],
                         start=(c == 0), stop=(c == CHT - 1))
    nc.vector.tensor_add(xn[:, 512:H], xps2[:], br_sb[:, 512:H])


# ----------------------------------------------------------------------------
# Launch A: conv1 + x1(bf16) + qw2 production.
# ----------------------------------------------------------------------------

def _build_A(prep):
    wx, bpc = prep["wx"], prep["bpc"]
    own_off = prep["own_off"]               # window tile index of first own block
    nxt = prep["nxt"]                       # source-window tiles
    abt = int(prep["slot_off"][-1])         # total ab tiles
    nc = bacc.Bacc("TRN2", target_bir_lowering=False, debug=False,
                   enable_asserts=False, num_devices=N_CORES)
    xin = nc.dram_tensor("xin", [wx, H], F32, kind="ExternalInput")
    ab = nc.dram_tensor("ab", [128, abt * 128], F8E4, kind="ExternalInput")
    wt = nc.dram_tensor("wt", [128, CHT * H], BF16, kind="ExternalInput")
    br = nc.dram_tensor("br", [128, H], F32, kind="ExternalInput")
    ts = nc.dram_tensor("ts", [128, 1], F32, kind="ExternalInput")
    ts2 = nc.dram_tensor("ts2", [128, 1], F32, kind="ExternalInput")
    x1out = nc.dram_tensor("x1out", [bpc * 128, H], BF16, kind="ExternalOutput")
    qw2out = nc.dram_tensor("qw2out", [bpc * 128, 2 * H], F8E4, kind="ExternalOutput")

    xin_r = xin.rearrange("(n p) d -> n p d", p=128)
    x1out_r = x1out.rearrange("(n p) d -> n p d", p=128)
    qw2out_r = qw2out.rearrange("(n p) d -> n p d", p=128)

    with tile.TileContext(nc) as tc:
        with (
            tc.tile_pool(name="persist", bufs=1) as pp,
            tc.tile_pool(name="epi", bufs=2) as ep,
            tc.tile_pool(name="psA", bufs=2, space="PSUM") as psA,
            tc.tile_pool(name="psW", bufs=1, space="PSUM") as psW,
        ):
            xw = pp.tile([128, nxt * H], F32)            # full f32 window
            rb = pp.tile([128, nxt * H], BF16)           # r' = relu(x)/16
            qw = pp.tile([128, nxt * 2 * H], F8E4)       # [q' | w] per window tile
            ab_sb = pp.tile([128, abt * 128], F8E4)
            wt_sb = pp.tile([128, CHT * H], BF16)
            br_sb = pp.tile([128, H], F32)
            ts_sb = pp.tile([128, 1], F32)
            ts16 = pp.tile([128, 1], F32)
            ts216 = pp.tile([128, 1], F32)
            ts2_sb = pp.tile([128, 1], F32)
            ident = pp.tile([128, 128], BF16)
            nb4 = pp.tile([128, 1], F32)
            masks.make_identity(nc, ident[:])
            nc.gpsimd.memset(nb4[:], -4.0)
            nc.sync.dma_start(ts_sb[:], ts[:])
            nc.sync.dma_start(ts2_sb[:], ts2[:])
            nc.vector.tensor_scalar_mul(ts16[:], ts_sb[:], 16.0)
            nc.vector.tensor_scalar_mul(ts216[:], ts2_sb[:], 16.0)

            # window DMAs first (they gate the elementwise chain and SpMM);
            # ab interleaved per quarter slab; weights/bias later.
            abw = abt * 128
            q4 = (abw // 4) // 128 * 128
            absl = [(0, q4), (q4, 2 * q4), (2 * q4, 3 * q4), (3 * q4, abw)]
            nc.sync.dma_start(ab_sb[:, absl[0][0]:absl[0][1]],
                              ab[:, absl[0][0]:absl[0][1]])
            for s in range(nxt):
                nc.sync.dma_start(xw[:, s * H:(s + 1) * H], xin_r[s])
                if s in (2, 5, 8):
                    i = s // 3 + 1
                    nc.sync.dma_start(ab_sb[:, absl[i][0]:absl[i][1]],
                                      ab[:, absl[i][0]:absl[i][1]])
                if s == 11:
                    nc.sync.dma_start(wt_sb[:], wt[:])
                if s == 13:
                    nc.sync.dma_start(br_sb[:], br[:])

            # window pass (2 tiles per op):
            # r' = relu(x)/16 (bf16), w = exp(16t*r') (fp8), q' = r'*w (fp8)
            for s in range(0, nxt, 2):
                pair = min(2, nxt - s)
                xt = xw[:, s * H:(s + pair) * H].rearrange(
                    "p (a d) -> p a d", d=H)
                rt = rb[:, s * H:(s + pair) * H].rearrange(
                    "p (a d) -> p a d", d=H)
                qv = qw[:, 2 * s * H:2 * (s + pair) * H].rearrange(
                    "p (a d) -> p a d", d=2 * H)
                qs, ws = qv[:, :, 0:H], qv[:, :, H:2 * H]
                if s % 4 == 0:
                    nc.vector.tensor_scalar(rt, xt, 0.0, 1.0 / 16.0,
                                            ALU.max, ALU.mult)
                else:
                    nc.scalar.activation(rt, xt, AF.Relu, scale=1.0 / 16.0)
                nc.scalar.activation(ws, rt, AF.Exp, scale=ts16[:, 0:1])
                nc.vector.tensor_mul(qs, rt, ws)

            # software pipeline: SpMM(bl) issued before epilogue(bl-1)
            aggs = [None] * bpc
            for bl in range(bpc + 1):
                if bl < bpc:
                    agg = psA.tile([128, 2 * H], F32, tag="agg")
                    _spmm_block(nc, agg, ab_sb, qw, bl, prep)
                    aggs[bl] = agg
                if bl >= 1:
                    pb = bl - 1
                    xo = xw[:, (own_off + pb) * H:(own_off + pb + 1) * H]
                    m_bf = _div_res(nc, ep, aggs[pb], xo)
                    xn = ep.tile([128, H], F32, tag="xn")
                    _mlp_block(nc, psW, ep, m_bf, wt_sb, ident, br_sb, xn)
                    x1b = ep.tile([128, H], BF16, tag="x1b")
                    nc.vector.tensor_copy(x1b[:], xn[:])
                    nc.sync.dma_start(x1out_r[pb], x1b[:])
                    # conv2 message tensors for the own rows (fp8, scaled):
                    # r2' = relu(x1)/16, w2 = exp(16*t2*r2' - 4), q2' = r2'*w2
                    # (the e^-4 shift and /16 scale cancel in the softmax)
                    r2b = ep.tile([128, H], BF16, tag="r2b")
                    nc.scalar.activation(r2b[:], xn[:], AF.Relu, scale=1.0 / 16.0)
                    qw2 = ep.tile([128, 2 * H], F8E4, tag="qw2")
                    nc.scalar.activation(qw2[:, H:2 * H], r2b[:], AF.Exp,
                                         scale=ts216[:, 0:1], bias=nb4[:, 0:1])
                    nc.vector.tensor_mul(qw2[:, 0:H], r2b[:], qw2[:, H:2 * H])
                    nc.sync.dma_start(qw2out_r[pb], qw2[:])
    nc.compile()
    return nc


# ----------------------------------------------------------------------------
# Launch B: conv2 + LN + colsums + partial Wc matvec.
# ----------------------------------------------------------------------------

def _build_B(prep, ln_trivial):
    wx, bpc = prep["wx"], prep["bpc"]
    nxt = prep["nxt"]
    abt = int(prep["slot_off"][-1])
    nc = bacc.Bacc("TRN2", target_bir_lowering=False, debug=False,
                   enable_asserts=False, num_devices=N_CORES)
    qwin = nc.dram_tensor("qwin", [wx, 2 * H], F8E4, kind="ExternalInput")
    x1own = nc.dram_tensor("x1own", [bpc * 128, H], BF16, kind="ExternalInput")
    ab = nc.dram_tensor("ab", [128, abt * 128], F8E4, kind="ExternalInput")
    wt = nc.dram_tensor("wt", [128, CHT * H], BF16, kind="ExternalInput")
    br = nc.dram_tensor("br", [128, H], F32, kind="ExternalInput")
    wct = nc.dram_tensor("wct", [128, 2 * CHT * H], BF16, kind="ExternalInput")
    if not ln_trivial:
        lngr = nc.dram_tensor("lngr", [128, H], F32, kind="ExternalInput")
        lnbr = nc.dram_tensor("lnbr", [128, H], F32, kind="ExternalInput")
    gpart = nc.dram_tensor("gpart", [1, H], F32, kind="ExternalOutput")

    qwin_r = qwin.rearrange("(n p) d -> n p d", p=128)
    x1_r = x1own.rearrange("(n p) d -> n p d", p=128)

    with tile.TileContext(nc) as tc:
        with (
            tc.tile_pool(name="persist", bufs=1) as pp,
            tc.tile_pool(name="epi", bufs=2) as ep,
            tc.tile_pool(name="psA", bufs=2, space="PSUM") as psA,
            tc.tile_pool(name="psW", bufs=1, space="PSUM") as psW,
            tc.tile_pool(name="psC", bufs=1, space="PSUM") as psC,
        ):
            qw = pp.tile([128, nxt * 2 * H], F8E4)
            x1_sb = pp.tile([128, bpc * H], BF16)
            ab_sb = pp.tile([128, abt * 128], F8E4)
            wt_sb = pp.tile([128, CHT * H], BF16)
            br_sb = pp.tile([128, H], F32)
            wct_sb = pp.tile([128, 2 * CHT * H], BF16)
            ident = pp.tile([128, 128], BF16)
            ones = pp.tile([128, 1], BF16)
            cs_sb = pp.tile([128, 2 * CHT], F32)
            lneps = pp.tile([128, 1], F32)
            masks.make_identity(nc, ident[:])
            nc.gpsimd.memset(ones[:], 1.0)
            nc.gpsimd.memset(cs_sb[:], 0.0)
            nc.gpsimd.memset(lneps[:], LN_EPS)

            # DMA order: ab slab (gates SpMM) interleaved with window tiles;
            # x1/wt/br next; wct (needed only at the end) last.
            abw = abt * 128
            q4 = (abw // 4) // 128 * 128
            absl = [(0, q4), (q4, 2 * q4), (2 * q4, 3 * q4), (3 * q4, abw)]
            nc.sync.dma_start(ab_sb[:, absl[0][0]:absl[0][1]],
                              ab[:, absl[0][0]:absl[0][1]])
            for s in range(nxt):
                nc.sync.dma_start(qw[:, s * 2 * H:(s + 1) * 2 * H], qwin_r[s])
                if s in (2, 5, 8):
                    i = s // 3 + 1
                    nc.sync.dma_start(ab_sb[:, absl[i][0]:absl[i][1]],
                                      ab[:, absl[i][0]:absl[i][1]])
            for bl in range(bpc):
                nc.sync.dma_start(x1_sb[:, bl * H:(bl + 1) * H], x1_r[bl])
            nc.sync.dma_start(wt_sb[:], wt[:])
            nc.sync.dma_start(br_sb[:], br[:])
            if not ln_trivial:
                lng_sb = pp.tile([128, H], F32)
                lnb_sb = pp.tile([128, H], F32)
                nc.sync.dma_start(lng_sb[:], lngr[:])
                nc.sync.dma_start(lnb_sb[:], lnbr[:])
            nc.sync.dma_start(wct_sb[:], wct[:])

            aggs = [None] * bpc
            for bl in range(bpc + 1):
                if bl < bpc:
                    agg = psA.tile([128, 2 * H], F32, tag="agg")
                    _spmm_block(nc, agg, ab_sb, qw, bl, prep)
                    aggs[bl] = agg
                if bl < 1:
                    continue
                pb = bl - 1
                xo = x1_sb[:, pb * H:(pb + 1) * H]
                m_bf = _div_res(nc, ep, aggs[pb], xo)
                xn = ep.tile([128, H], F32, tag="xn")
                _mlp_block(nc, psW, ep, m_bf, wt_sb, ident, br_sb, xn)

                # LayerNorm stats via bn_stats (3 x 256 subgroups)
                stats = ep.tile([128, 3, 6], F32, tag="stats")
                xn_g = xn[:].rearrange("p (a b) -> p a b", b=256)
                for g in range(3):
                    nc.vector.bn_stats(stats[:, g, :], xn_g[:, g, :])
                mv = ep.tile([128, 2], F32, tag="mv")
                nc.vector.bn_aggr(mv[:], stats[:])
                var = ep.tile([128, 1], F32, tag="var")
                nc.vector.tensor_scalar(var[:], mv[:, 1:2], lneps[:, 0:1], None,
                                        ALU.add)
                rstd = ep.tile([128, 1], F32, tag="rstd")
                nc.vector.reciprocal_approx_fast(rstd[:], var[:])
                nc.scalar.sqrt(rstd[:], rstd[:])
                nmr = ep.tile([128, 1], F32, tag="nmr")
                nc.vector.tensor_scalar(nmr[:], mv[:, 0:1], rstd[:, 0:1], -1.0,
                                        ALU.mult, ALU.mult)
                hr = ep.tile([128, H], BF16, tag="hr")
                if ln_trivial:
                    # ln_g == 1, ln_b == 0: relu(LN(x)) in one activation
                    nc.scalar.activation(hr[:], xn[:], AF.Relu,
                                         bias=nmr[:, 0:1], scale=rstd[:, 0:1])
                else:
                    hn = ep.tile([128, H], F32, tag="hn")
                    nc.scalar.activation(hn[:], xn[:], AF.Identity,
                                         bias=nmr[:, 0:1], scale=rstd[:, 0:1])
                    nc.vector.tensor_mul(hn[:], hn[:], lng_sb[:])
                    nc.vector.tensor_add(hn[:], hn[:], lnb_sb[:])
                    nc.scalar.activation(hr[:], hn[:], AF.Relu)

                # channel-major column sums off bf16 stationaries:
                # cs[:, 0:6] += colsum(x1_blk), cs[:, 6:12] += colsum(hr_blk)
                cs_ps = psC.tile([128, 2 * CHT], F32, tag="cs")
                for c in range(CHT):
                    nc.tensor.matmul(cs_ps[:, c:c + 1], xo[:, c * 128:(c + 1) * 128],
                                     ones[:], start=True, stop=True)
                    nc.tensor.matmul(cs_ps[:, CHT + c:CHT + c + 1],
                                     hr[:, c * 128:(c + 1) * 128],
                                     ones[:], start=True, stop=True)
                nc.vector.tensor_add(cs_sb[:], cs_sb[:], cs_ps[:])

            # cs2 = cs_x1 + cs_hr ; bf16 for the matvec
            csb = pp.tile([128, 2 * CHT], BF16)
            nc.vector.tensor_copy(csb[:, 0:CHT], cs_sb[:, 0:CHT])
            nc.vector.tensor_add(csb[:, CHT:2 * CHT], cs_sb[:, 0:CHT],
                                 cs_sb[:, CHT:2 * CHT])
            # per-core partial g = cs_c @ Wc.T (unscaled; bf16 matvec, 2 passes)
            gout = pp.tile([1, H], F32)
            for h in range(2):                       # 2 x 384 output columns
                g_ps = psW.tile([1, 384], F32, tag="pw")
                for j in range(2 * CHT):
                    nc.tensor.matmul(g_ps[:], csb[:, j:j + 1],
                                     wct_sb[:, j * H + h * 384:j * H + (h + 1) * 384],
                                     start=(j == 0), stop=(j == 2 * CHT - 1))
                nc.vector.tensor_copy(gout[:, h * 384:(h + 1) * 384], g_ps[:])
            nc.sync.dma_start(gpart[:], gout[:])
    nc.compile()
    return nc


# ----------------------------------------------------------------------------
# Launch C: matmul-free finalize, channel-major [128, CHT] layout.
# row0_cm = sum_c parts_c / n + bc_cm + x0_cm
# ----------------------------------------------------------------------------

def _build_C(n):
    nc = bacc.Bacc("TRN2", target_bir_lowering=False, debug=False,
                   enable_asserts=False, num_devices=N_CORES)
    # parts_cm[p, j*N_CORES + c] = gpart_c[j*128 + p]
    parts = nc.dram_tensor("parts", [128, CHT * N_CORES], F32, kind="ExternalInput")
    bcr = nc.dram_tensor("bcr", [128, CHT], F32, kind="ExternalInput")
    x0r = nc.dram_tensor("x0r", [128, CHT], F32, kind="ExternalInput")
    row0 = nc.dram_tensor("row0", [128, CHT], F32, kind="ExternalOutput")

    with tile.TileContext(nc) as tc:
        with tc.tile_pool(name="sb", bufs=1) as sb:
            pt = sb.tile([128, CHT * N_CORES], F32)
            bc_sb = sb.tile([128, CHT], F32)
            x0_sb = sb.tile([128, CHT], F32)
            nc.sync.dma_start(pt[:], parts[:])
            nc.sync.dma_start(bc_sb[:], bcr[:])
            nc.sync.dma_start(x0_sb[:], x0r[:])
            red = sb.tile([128, CHT], F32)
            nc.vector.tensor_reduce(
                red[:], pt[:].rearrange("p (j c) -> p j c", c=N_CORES),
                mybir.AxisListType.X, ALU.add)
            out_sb = sb.tile([128, CHT], F32)
            nc.vector.tensor_scalar(out_sb[:], red[:], 1.0 / 4096.0, None,
                                    ALU.mult)
            nc.vector.tensor_add(out_sb[:], out_sb[:], bc_sb[:])
            nc.vector.tensor_add(out_sb[:], out_sb[:], x0_sb[:])
            nc.sync.dma_start(row0[:], out_sb[:])
    nc.compile()
    return nc


def _pack_wt(w, dtype=np.float32):
    """[Hout, Hin] weight -> partition-major packed W.T tiles [128, (Hin/128)*Hout]:
    out[p, c*Hout + o] = W[o, c*128 + p]"""
    h_out, h_in = w.shape
    nt = h_in // 128
    out = np.empty((128, nt * h_out), dtype=np.float32)
    for c in range(nt):
        out[:, c * h_out:(c + 1) * h_out] = w[:, c * 128:(c + 1) * 128].T
    return np.ascontiguousarray(out.astype(dtype))


def _to_cm(v):
    """[768] -> channel-major [128, 6]: out[p, j] = v[j*128+p]."""
    return np.ascontiguousarray(v.reshape(CHT, 128).T.astype(np.float32))


def kernel(**inputs):
    x = np.asarray(inputs["x"], dtype=np.float32)
    w1 = np.asarray(inputs["W1"], dtype=np.float32)
    b1 = np.asarray(inputs["b1"], dtype=np.float32)
    t1 = np.float32(np.asarray(inputs["t1"]))
    w2 = np.asarray(inputs["W2"], dtype=np.float32)
    b2 = np.asarray(inputs["b2"], dtype=np.float32)
    t2 = np.float32(np.asarray(inputs["t2"]))
    ln_g = np.asarray(inputs["ln_g"], dtype=np.float32)
    ln_b = np.asarray(inputs["ln_b"], dtype=np.float32)
    wc = np.asarray(inputs["Wc"], dtype=np.float32)
    bc = np.asarray(inputs["bc"], dtype=np.float32)
    ei = np.asarray(inputs["edge_index"])

    n = x.shape[1]
    ln_trivial = bool(np.all(ln_g == 1.0) and np.all(ln_b == 0.0))
    ekey = (ei.shape[1], n, ln_trivial,
            int(np.bitwise_xor.reduce(ei[0].astype(np.int64) * 31 + ei[1])))
    if ekey not in _cache:
        prep = _prepare(ei, n)
        progs = dict(A=_build_A(prep), B=_build_B(prep, ln_trivial),
                     C=_build_C(n))
        _cache[ekey] = (prep, progs)
    prep, progs = _cache[ekey]
    perm, bpc = prep["perm"], prep["bpc"]

    xp = np.ascontiguousarray(x[0][perm])            # permuted node features
    t1r = np.full((128, 1), t1, dtype=np.float32)
    t2r = np.full((128, 1), t2, dtype=np.float32)
    w1t = _pack_wt(w1, ml_dtypes.bfloat16)
    w2t = _pack_wt(w2, ml_dtypes.bfloat16)
    wct = _pack_wt(wc, ml_dtypes.bfloat16)
    b1r = np.ascontiguousarray(np.broadcast_to(b1, (128, H)))
    b2r = np.ascontiguousarray(np.broadcast_to(b2, (128, H)))
    lngr = np.ascontiguousarray(np.broadcast_to(ln_g, (128, H)))
    lnbr = np.ascontiguousarray(np.broadcast_to(ln_b, (128, H)))

    cores = list(range(N_CORES))

    # --- launch A: conv1 -> x1(bf16) + qw2 ---
    mapsA = [dict(xin=_win_slice(xp, prep, c), ab=prep["abands"][c],
                  wt=w1t, br=b1r, ts=t1r, ts2=t2r) for c in cores]
    resA = run_bass_kernel_spmd(progs["A"], mapsA, core_ids=cores)
    x1 = np.concatenate([resA.results[c]["x1out"] for c in cores], axis=0)
    qw2 = np.concatenate([resA.results[c]["qw2out"] for c in cores], axis=0)

    # --- launch B: conv2 + LN + colsums + partial Wc matvec ---
    mapsB = []
    for c in cores:
        m = dict(qwin=_win_slice(qw2, prep, c),
                 x1own=x1[c * bpc * 128:(c + 1) * bpc * 128],
                 ab=prep["abands"][c], wt=w2t, br=b2r, wct=wct)
        if not ln_trivial:
            m["lngr"] = lngr
            m["lnbr"] = lnbr
        mapsB.append(m)
    resB = run_bass_kernel_spmd(progs["B"], mapsB, core_ids=cores)
    g = np.stack([resB.results[c]["gpart"][0] for c in cores])   # [8, 768]
    # channel-major stack: parts_cm[p, j*8+c] = g[c, j*128+p]
    parts_cm = np.ascontiguousarray(
        g.reshape(N_CORES, CHT, 128).transpose(2, 1, 0).reshape(128, CHT * N_CORES))

    # --- launch C: finalize row0 ---
    mapsC = [dict(parts=parts_cm, bcr=_to_cm(bc), x0r=_to_cm(x[0, 0]))
             for _ in cores]
    resC = run_bass_kernel_spmd(progs["C"], mapsC, core_ids=cores)
    row0 = resC.results[0]["row0"].T.reshape(H)      # channel-major -> [768]

    out = x.copy()
    out[0, 0, :] = row0
    return out


# revision 35
# speedup vs baseline: 1.0034x; 1.0034x over previous
"""Trainium2 Bass kernel for a 2-layer GENConv (softmax aggr) + LayerNorm GNN block.

Distribution: graph-partitioned across 8 NeuronCores. Nodes are reordered by a
Fiedler-vector (spectral 1D) layout so the adjacency becomes banded; the
per-channel softmax aggregation collapses to two banded-SpMM matmuls because
GENConv softmax logits depend only on the source node:

  r = relu(x); w = exp(t*r); q = w*r
  num = A @ q;  den = A @ w;  agg = num/den        (the max-shift cancels; the
  1e-7 message eps shifts agg by exactly 1e-7 — far below tolerance — dropped)

Each core owns 4 contiguous dst blocks of 128 nodes; its banded A^T slab and
the qw window it contracts against are uniform across cores (SPMD), with
per-core variation expressed purely through input data (zero-padded bands).

Three SPMD launches (host work between them is pure data movement):
  A: conv1, software-pipelined (SpMM of block b+1 issued before the epilogue
     of block b so the PE never drains); emits x1 in bf16 and conv2's message
     tensors q2|w2 = relu(x1)*exp(t2*relu(x1)), exp(t2*relu(x1)).
  B: conv2 from the precomputed qw2 window (no window elementwise at all),
     LayerNorm via bn_stats, channel-major column sums off bf16 tiles
     (cs2 = cs_x1 + cs_relu(LN) — x2 itself is never materialized), and a
     per-core partial Wc matvec g_c = colsums_c @ Wc.T (bf16).
  C: tiny matmul-free finalize in channel-major [128,6] layout:
     row0 = sum_c g_c / n + bc + x0.
"""

import ml_dtypes
import numpy as np

import concourse.bass as bass
import concourse.bacc as bacc
import concourse.mybir as mybir
import concourse.tile as tile
import concourse.masks as masks
from concourse.bass_utils import run_bass_kernel_spmd

F32 = mybir.dt.float32
BF16 = mybir.dt.bfloat16
F8E4 = mybir.dt.float8e4
AF = mybir.ActivationFunctionType
ALU = mybir.AluOpType

N_CORES = 8
H = 768
CHT = H // 128           # channel tiles = 6
LN_EPS = 1e-5

_cache = {}


# ----------------------------------------------------------------------------
# Host-side graph preprocessing (index work only — no float math on data).
# ----------------------------------------------------------------------------

def _band_struct(ns, nd, n, bpc):
    """Per-block source-tile extents [st, en) (in 128-tiles) of the permuted
    adjacency, and the per-slot window ranges shared across cores."""
    nb = n // 128
    order = np.lexsort((ns, nd))
    ns_s, nd_s = ns[order], nd[order]
    starts = np.searchsorted(nd_s, np.arange(0, n, 128))
    ends = np.searchsorted(nd_s, np.arange(128, n + 1, 128))
    st = np.empty(nb, dtype=np.int64)
    en = np.empty(nb, dtype=np.int64)
    for b in range(nb):
        s = ns_s[starts[b]:ends[b]]
        st[b] = s.min() // 128
        en[b] = s.max() // 128 + 1
    pmax_t = int((np.arange(nb) - st).max())         # tiles left of own block
    # window origin (tile) for core c is c*bpc - pmax_t; slot ranges are the
    # max hull across cores in window-tile coordinates
    slot_lo, slot_hi = [], []
    for s in range(bpc):
        org = np.arange(N_CORES) * bpc - pmax_t
        blocks = np.arange(N_CORES) * bpc + s
        slot_lo.append(int((st[blocks] - org).min()))
        slot_hi.append(int((en[blocks] - org).max()))
    wlo = min(slot_lo)
    whi = max(slot_hi)
    slot_lo = [lo - wlo for lo in slot_lo]
    slot_hi = [hi - wlo for hi in slot_hi]
    return st, en, pmax_t, wlo, whi, slot_lo, slot_hi


def _ordering(src, dst, n, bpc):
    """1D layout minimizing the per-slot banded-SpMM tile count: scan linear
    directions in the span of Laplacian eigenvectors 1..3 (the near-degenerate
    first modes of a 3D point cloud mix arbitrarily; a pure axis mode gives
    the narrowest band). Falls back to the Fiedler vector / identity."""
    import scipy.sparse as sp
    a = sp.csr_matrix(
        (np.ones(len(src), dtype=np.float64), (dst, src)), shape=(n, n)
    )
    asym = ((a + a.T) > 0).astype(np.float64)
    try:
        from scipy.sparse.linalg import eigsh
        lap = sp.diags(np.asarray(asym.sum(1)).ravel()) - asym
        _, vecs = eigsh(lap, k=4, sigma=-1e-4, which="LM")
        emb = vecs[:, 1:4]
    except Exception:
        return np.arange(n, dtype=np.int64)

    inv = np.empty(n, dtype=np.int64)

    def cost(perm):
        inv[perm] = np.arange(n)
        _, _, _, wlo, whi, slot_lo, slot_hi = _band_struct(
            inv[src], inv[dst], n, bpc)
        return (sum(hi - lo for lo, hi in zip(slot_lo, slot_hi)), whi - wlo)

    rngs = np.random.RandomState(42)
    dirs = [np.eye(3)[i] for i in range(3)]
    dirs += [v / np.linalg.norm(v) for v in rngs.randn(240, 3)]
    best = None
    for u in dirs:
        perm = np.argsort(emb @ u).astype(np.int64)
        c = cost(perm)
        if best is None or c < best[0]:
            best = (c, perm)
    return best[1]


def _prepare(edge_index, n):
    import scipy.sparse as sp
    src = np.asarray(edge_index[0], dtype=np.int64)
    dst = np.asarray(edge_index[1], dtype=np.int64)
    nb = n // 128
    bpc = nb // N_CORES                     # blocks per core
    perm = _ordering(src, dst, n, bpc)      # new position i holds old node perm[i]
    inv = np.empty(n, dtype=np.int64)
    inv[perm] = np.arange(n)
    ns, nd = inv[src], inv[dst]             # edges in new coordinates

    st, en, pmax_t, wlo, whi, slot_lo, slot_hi = _band_struct(ns, nd, n, bpc)
    slot_S = [hi - lo for lo, hi in zip(slot_lo, slot_hi)]
    slot_off = np.concatenate([[0], np.cumsum(slot_S)]).astype(int)
    nxt = whi - wlo                          # window tiles per core
    own_off = pmax_t - wlo                   # window tile of first own block
    wx = nxt * 128

    # banded A^T slabs, packed per (slot, k) for contiguous DMA:
    # ab[c][p, (slot_off[s]+k)*128 + d] =
    #   #edges src=(win_org + (slot_lo[s]+k)*128 + p) -> dst=(blk(c,s)*128 + d)
    amat = sp.csr_matrix(
        (np.ones(len(ns), dtype=np.float64), (nd, ns)), shape=(n, n))
    abands = []
    for c in range(N_CORES):
        org = (c * bpc - pmax_t + wlo) * 128          # window row origin
        ab = np.zeros((128, slot_off[-1] * 128), dtype=np.float32)
        for s in range(bpc):
            blk = (c * bpc + s) * 128
            for k in range(slot_S[s]):
                r0 = org + (slot_lo[s] + k) * 128     # src rows of this tile
                a0, a1 = max(r0, 0), min(r0 + 128, n)
                if a0 >= a1:
                    continue
                sub = np.asarray(
                    amat[blk:blk + 128, a0:a1].todense(), dtype=np.float32)
                tilecol = (slot_off[s] + k) * 128
                ab[a0 - r0:a1 - r0, tilecol:tilecol + 128] = sub.T
        abands.append(ab.astype(ml_dtypes.float8_e4m3))

    return dict(perm=perm, inv=inv, pmax_t=pmax_t, wlo=wlo, nxt=nxt, wx=wx,
                own_off=own_off, slot_lo=slot_lo, slot_S=slot_S,
                slot_off=slot_off, bpc=bpc, abands=abands)


def _win_slice(full, prep, c):
    """Window rows of `full` for core c, zero-padded."""
    n = full.shape[0]
    bpc, pmax_t, wlo, wx = prep["bpc"], prep["pmax_t"], prep["wlo"], prep["wx"]
    lo = (c * bpc - pmax_t + wlo) * 128
    hi = lo + wx
    out = np.zeros((hi - lo, full.shape[1]), dtype=full.dtype)
    a, b = max(lo, 0), min(hi, n)
    out[a - lo:b - lo] = full[a:b]
    return out


# ----------------------------------------------------------------------------
# Shared Bass fragments.
# ----------------------------------------------------------------------------

def _spmm_block(nc, agg, ab_sb, qw, bl, prep):
    """agg[128,2H] (PSUM) += banded A^T slab tiles x qw window tiles.
    fp8 DoubleRow: each matmul contracts a PAIR of 128-row k-tiles
    (lhsT [128,2,128], rhs [128,2,512]) at 2x PE rate; odd tail single."""
    S = prep["slot_S"][bl]
    off = prep["slot_off"][bl]
    lo = prep["slot_lo"][bl]
    k = 0
    while k < S:
        pair = 2 if k + 1 < S else 1
        s = lo + k                      # first window tile of this pair
        last = k + pair >= S
        if pair == 2:
            at = ab_sb[:, (off + k) * 128:(off + k + 2) * 128].rearrange(
                "p (j d) -> p j d", d=128)
            qp = qw[:, s * 2 * H:(s + 2) * 2 * H].rearrange(
                "p (j d) -> p j d", d=2 * H)
            for ch in range(3):         # 1536 free = 3 x 512
                qv = qp[:, :, ch * 512:(ch + 1) * 512]
                nc.tensor.matmul(
                    agg[:, ch * 512:(ch + 1) * 512], at, qv,
                    start=(k == 0), stop=last,
                    perf_mode=mybir.MatmulPerfMode.DoubleRow,
                )
        else:
            at = ab_sb[:, (off + k) * 128:(off + k + 1) * 128]
            for ch in range(3):
                nc.tensor.matmul(
                    agg[:, ch * 512:(ch + 1) * 512], at,
                    qw[:, s * 2 * H + ch * 512:s * 2 * H + (ch + 1) * 512],
                    start=(k == 0), stop=last,
                )
        k += pair


def _div_res(nc, ep, agg, xo):
    """m_bf (bf16) = 16 * agg[:, :H] / agg[:, H:] + xo  (softmax divide +
    residual; the 16 undoes the q/16 fp8-range scaling of the messages).
    Runs in two half-width passes so the first transposes can start ~1.5us
    earlier (shorter critical chain on the last block)."""
    rec = ep.tile([128, H], F32, tag="rec")
    mtmp = ep.tile([128, H], F32, tag="mtmp")
    m_bf = ep.tile([128, H], BF16, tag="m_bf")
    hh = H // 2
    for i in range(2):
        sl = slice(i * hh, (i + 1) * hh)
        nc.vector.reciprocal_approx_fast(rec[:, sl], agg[:, H + i * hh:H + (i + 1) * hh])
        nc.vector.tensor_mul(mtmp[:, sl], agg[:, sl], rec[:, sl])
        nc.vector.scalar_tensor_tensor(m_bf[:, sl], mtmp[:, sl], 16.0, xo[:, sl],
                                       ALU.mult, ALU.add)
    return m_bf


def _mlp_block(nc, pW, ep, m_bf, wt_sb, ident, br_sb, xn):
    """xn[128,H] (SBUF f32) = m_bf @ W.T + b via 6 transposes + 12 matmuls.
    pW is a single-bank PSUM pool reused for the transposes and both
    output passes (sequential requests serialize safely)."""
    tp = pW.tile([128, H], BF16, tag="pw")
    for c in range(CHT):
        nc.tensor.transpose(tp[:, c * 128:(c + 1) * 128],
                            m_bf[:, c * 128:(c + 1) * 128], ident[:])
    mt = ep.tile([128, H], BF16, tag="mt")
    for c in range(CHT):
        nc.scalar.copy(mt[:, c * 128:(c + 1) * 128], tp[:, c * 128:(c + 1) * 128])
    xps1 = pW.tile([128, 512], F32, tag="pw")
    for c in range(CHT):
        nc.tensor.matmul(xps1[:], mt[:, c * 128:(c + 1) * 128],
                         wt_sb[:, c * H:c * H + 512],
                         start=(c == 0), stop=(c == CHT - 1))
    nc.vector.tensor_add(xn[:, 0:512], xps1[:], br_sb[:, 0:512])
    xps2 = pW.tile([128, 256], F32, tag="pw")
    for c in range(CHT):
        nc.tensor.matmul(xps2[:], mt[:, c * 128:(c + 1) * 128],
                         wt_sb[:, c * H + 512:(c + 1) * H],
                         start=(c == 0), stop=(c == CHT - 1))
    nc.vector.tensor_add(xn[:, 512:H], xps2[:], br_sb[:, 512:H])


# ----------------------------------------------------------------------------
# Launch A: conv1 + x1(bf16) + qw2 production.
# ----------------------------------------------------------------------------

def _build_A(prep):
    wx, bpc = prep["wx"], prep["bpc"]
    own_off = prep["own_off"]               # window tile index of first own block
    nxt = prep["nxt"]                       # source-window tiles
    abt = int(prep["slot_off"][-1])         # total ab tiles
    nc = bacc.Bacc("TRN2", target_bir_lowering=False, debug=False,
                   enable_asserts=False, num_devices=N_CORES)
    xin = nc.dram_tensor("xin", [wx, H], F32, kind="ExternalInput")
    ab = nc.dram_tensor("ab", [128, abt * 128], F8E4, kind="ExternalInput")
    wt = nc.dram_tensor("wt", [128, CHT * H], BF16, kind="ExternalInput")
    br = nc.dram_tensor("br", [128, H], F32, kind="ExternalInput")
    ts = nc.dram_tensor("ts", [128, 1], F32, kind="ExternalInput")
    ts2 = nc.dram_tensor("ts2", [128, 1], F32, kind="ExternalInput")
    x1out = nc.dram_tensor("x1out", [bpc * 128, H], BF16, kind="ExternalOutput")
    qw2out = nc.dram_tensor("qw2out", [bpc * 128, 2 * H], F8E4, kind="ExternalOutput")

    xin_r = xin.rearrange("(n p) d -> n p d", p=128)
    x1out_r = x1out.rearrange("(n p) d -> n p d", p=128)
    qw2out_r = qw2out.rearrange("(n p) d -> n p d", p=128)

    with tile.TileContext(nc) as tc:
        with (
            tc.tile_pool(name="persist", bufs=1) as pp,
            tc.tile_pool(name="epi", bufs=2) as ep,
            tc.tile_pool(name="psA", bufs=2, space="PSUM") as psA,
            tc.tile_pool(name="psW", bufs=1, space="PSUM") as psW,
        ):
            xw = pp.tile([128, nxt * H], F32)            # full f32 window
            rb = pp.tile([128, nxt * H], BF16)           # r' = relu(x)/16
            qw = pp.tile([128, nxt * 2 * H], F8E4)       # [q' | w] per window tile
            ab_sb = pp.tile([128, abt * 128], F8E4)
            wt_sb = pp.tile([128, CHT * H], BF16)
            br_sb = pp.tile([128, H], F32)
            ts_sb = pp.tile([128, 1], F32)
            ts16 = pp.tile([128, 1], F32)
            ts216 = pp.tile([128, 1], F32)
            ts2_sb = pp.tile([128, 1], F32)
            ident = pp.tile([128, 128], BF16)
            nb4 = pp.tile([128, 1], F32)
            masks.make_identity(nc, ident[:])
            nc.gpsimd.memset(nb4[:], -4.0)
            nc.sync.dma_start(ts_sb[:], ts[:])
            nc.sync.dma_start(ts2_sb[:], ts2[:])
            nc.vector.tensor_scalar_mul(ts16[:], ts_sb[:], 16.0)
            nc.vector.tensor_scalar_mul(ts216[:], ts2_sb[:], 16.0)

            # window DMAs first (they gate the elementwise chain and SpMM);
            # ab interleaved per quarter slab; weights/bias later.
            abw = abt * 128
            q4 = (abw // 4) // 128 * 128
            absl = [(0, q4), (q4, 2 * q4), (2 * q4, 3 * q4), (3 * q4, abw)]
            nc.sync.dma_start(ab_sb[:, absl[0][0]:absl[0][1]],
                              ab[:, absl[0][0]:absl[0][1]])
            for s in range(nxt):
                nc.sync.dma_start(xw[:, s * H:(s + 1) * H], xin_r[s])
                if s in (2, 5, 8):
                    i = s // 3 + 1
                    nc.sync.dma_start(ab_sb[:, absl[i][0]:absl[i][1]],
                                      ab[:, absl[i][0]:absl[i][1]])
                if s == 11:
                    nc.sync.dma_start(wt_sb[:], wt[:])
                if s == 13:
                    nc.sync.dma_start(br_sb[:], br[:])

            # window pass (2 tiles per op):
            # r' = relu(x)/16 (bf16), w = exp(16t*r') (fp8), q' = r'*w (fp8)
            for s in range(0, nxt, 2):
                pair = min(2, nxt - s)
                xt = xw[:, s * H:(s + pair) * H].rearrange(
                    "p (a d) -> p a d", d=H)
                rt = rb[:, s * H:(s + pair) * H].rearrange(
                    "p (a d) -> p a d", d=H)
                qv = qw[:, 2 * s * H:2 * (s + pair) * H].rearrange(
                    "p (a d) -> p a d", d=2 * H)
                qs, ws = qv[:, :, 0:H], qv[:, :, H:2 * H]
                nc.scalar.activation(rt, xt, AF.Relu, scale=1.0 / 16.0)
                nc.scalar.activation(ws, rt, AF.Exp, scale=ts16[:, 0:1])
                nc.vector.tensor_mul(qs, rt, ws)

            # software pipeline: SpMM(bl) issued before epilogue(bl-1)
            aggs = [None] * bpc
            for bl in range(bpc + 1):
                if bl < bpc:
                    agg = psA.tile([128, 2 * H], F32, tag="agg")
                    _spmm_block(nc, agg, ab_sb, qw, bl, prep)
                    aggs[bl] = agg
                if bl >= 1:
                    pb = bl - 1
                    xo = xw[:, (own_off + pb) * H:(own_off + pb + 1) * H]
                    m_bf = _div_res(nc, ep, aggs[pb], xo)
                    xn = ep.tile([128, H], BF16, tag="xn")
                    _mlp_block(nc, psW, ep, m_bf, wt_sb, ident, br_sb, xn)
                    nc.sync.dma_start(x1out_r[pb], xn[:])
                    # conv2 message tensors for the own rows (fp8, scaled):
                    # r2' = relu(x1)/16, w2 = exp(16*t2*r2' - 4), q2' = r2'*w2
                    # (the e^-4 shift and /16 scale cancel in the softmax)
                    r2b = ep.tile([128, H], BF16, tag="r2b")
                    nc.scalar.activation(r2b[:], xn[:], AF.Relu, scale=1.0 / 16.0)
                    qw2 = ep.tile([128, 2 * H], F8E4, tag="qw2")
                    nc.scalar.activation(qw2[:, H:2 * H], r2b[:], AF.Exp,
                                         scale=ts216[:, 0:1], bias=nb4[:, 0:1])
                    nc.vector.tensor_mul(qw2[:, 0:H], r2b[:], qw2[:, H:2 * H])
                    nc.sync.dma_start(qw2out_r[pb], qw2[:])
    nc.compile()
    return nc


# ----------------------------------------------------------------------------
# Launch B: conv2 + LN + colsums + partial Wc matvec.
# ----------------------------------------------------------------------------

def _build_B(prep, ln_trivial):
    wx, bpc = prep["wx"], prep["bpc"]
    nxt = prep["nxt"]
    abt = int(prep["slot_off"][-1])
    nc = bacc.Bacc("TRN2", target_bir_lowering=False, debug=False,
                   enable_asserts=False, num_devices=N_CORES)
    qwin = nc.dram_tensor("qwin", [wx, 2 * H], F8E4, kind="ExternalInput")
    x1own = nc.dram_tensor("x1own", [bpc * 128, H], BF16, kind="ExternalInput")
    ab = nc.dram_tensor("ab", [128, abt * 128], F8E4, kind="ExternalInput")
    wt = nc.dram_tensor("wt", [128, CHT * H], BF16, kind="ExternalInput")
    br = nc.dram_tensor("br", [128, H], F32, kind="ExternalInput")
    wct = nc.dram_tensor("wct", [128, 2 * CHT * H], BF16, kind="ExternalInput")
    if not ln_trivial:
        lngr = nc.dram_tensor("lngr", [128, H], F32, kind="ExternalInput")
        lnbr = nc.dram_tensor("lnbr", [128, H], F32, kind="ExternalInput")
    gpart = nc.dram_tensor("gpart", [1, H], F32, kind="ExternalOutput")

    qwin_r = qwin.rearrange("(n p) d -> n p d", p=128)
    x1_r = x1own.rearrange("(n p) d -> n p d", p=128)

    with tile.TileContext(nc) as tc:
        with (
            tc.tile_pool(name="persist", bufs=1) as pp,
            tc.tile_pool(name="epi", bufs=2) as ep,
            tc.tile_pool(name="psA", bufs=2, space="PSUM") as psA,
            tc.tile_pool(name="psW", bufs=1, space="PSUM") as psW,
            tc.tile_pool(name="psC", bufs=1, space="PSUM") as psC,
        ):
            qw = pp.tile([128, nxt * 2 * H], F8E4)
            x1_sb = pp.tile([128, bpc * H], BF16)
            ab_sb = pp.tile([128, abt * 128], F8E4)
            wt_sb = pp.tile([128, CHT * H], BF16)
            br_sb = pp.tile([128, H], F32)
            wct_sb = pp.tile([128, 2 * CHT * H], BF16)
            ident = pp.tile([128, 128], BF16)
            ones = pp.tile([128, 1], BF16)
            cs_sb = pp.tile([128, 2 * CHT], F32)
            lneps = pp.tile([128, 1], F32)
            masks.make_identity(nc, ident[:])
            nc.gpsimd.memset(ones[:], 1.0)
            nc.gpsimd.memset(cs_sb[:], 0.0)
            nc.gpsimd.memset(lneps[:], LN_EPS)

            # DMA order: ab slab (gates SpMM) interleaved with window tiles;
            # x1/wt/br next; wct (needed only at the end) last.
            abw = abt * 128
            q4 = (abw // 4) // 128 * 128
            absl = [(0, q4), (q4, 2 * q4), (2 * q4, 3 * q4), (3 * q4, abw)]
            nc.sync.dma_start(ab_sb[:, absl[0][0]:absl[0][1]],
                              ab[:, absl[0][0]:absl[0][1]])
            for s in range(nxt):
                nc.sync.dma_start(qw[:, s * 2 * H:(s + 1) * 2 * H], qwin_r[s])
                if s in (2, 5, 8):
                    i = s // 3 + 1
                    nc.sync.dma_start(ab_sb[:, absl[i][0]:absl[i][1]],
                                      ab[:, absl[i][0]:absl[i][1]])
            for bl in range(bpc):
                nc.sync.dma_start(x1_sb[:, bl * H:(bl + 1) * H], x1_r[bl])
            nc.sync.dma_start(wt_sb[:], wt[:])
            nc.sync.dma_start(br_sb[:], br[:])
            if not ln_trivial:
                lng_sb = pp.tile([128, H], F32)
                lnb_sb = pp.tile([128, H], F32)
                nc.sync.dma_start(lng_sb[:], lngr[:])
                nc.sync.dma_start(lnb_sb[:], lnbr[:])
            nc.sync.dma_start(wct_sb[:], wct[:])

            aggs = [None] * bpc
            hrs = [None] * bpc
            for bl in range(bpc + 2):
                if bl < bpc:
                    agg = psA.tile([128, 2 * H], F32, tag="agg")
                    _spmm_block(nc, agg, ab_sb, qw, bl, prep)
                    aggs[bl] = agg
                if 1 <= bl <= bpc:
                    pb = bl - 1
                    xo = x1_sb[:, pb * H:(pb + 1) * H]
                    m_bf = _div_res(nc, ep, aggs[pb], xo)
                    xn = ep.tile([128, H], F32, tag="xn")
                    _mlp_block(nc, psW, ep, m_bf, wt_sb, ident, br_sb, xn)

                    # LayerNorm stats via bn_stats (3 x 256 subgroups)
                    stats = ep.tile([128, 3, 6], F32, tag="stats")
                    xn_g = xn[:].rearrange("p (a b) -> p a b", b=256)
                    for g in range(3):
                        nc.vector.bn_stats(stats[:, g, :], xn_g[:, g, :])
                    mv = ep.tile([128, 2], F32, tag="mv")
                    nc.vector.bn_aggr(mv[:], stats[:])
                    var = ep.tile([128, 1], F32, tag="var")
                    nc.vector.tensor_scalar(var[:], mv[:, 1:2], lneps[:, 0:1],
                                            None, ALU.add)
                    rstd = ep.tile([128, 1], F32, tag="rstd")
                    nc.vector.reciprocal_approx_fast(rstd[:], var[:])
                    nc.scalar.sqrt(rstd[:], rstd[:])
                    nmr = ep.tile([128, 1], F32, tag="nmr")
                    nc.vector.tensor_scalar(nmr[:], mv[:, 0:1], rstd[:, 0:1],
                                            -1.0, ALU.mult, ALU.mult)
                    hr = ep.tile([128, H], BF16, tag="hr")
                    if ln_trivial:
                        # ln_g == 1, ln_b == 0: relu(LN(x)) in one activation
                        nc.scalar.activation(hr[:], xn[:], AF.Relu,
                                             bias=nmr[:, 0:1], scale=rstd[:, 0:1])
                    else:
                        hn = ep.tile([128, H], F32, tag="hn")
                        nc.scalar.activation(hn[:], xn[:], AF.Identity,
                                             bias=nmr[:, 0:1], scale=rstd[:, 0:1])
                        nc.vector.tensor_mul(hn[:], hn[:], lng_sb[:])
                        nc.vector.tensor_add(hn[:], hn[:], lnb_sb[:])
                        nc.scalar.activation(hr[:], hn[:], AF.Relu)
                    hrs[pb] = hr
                if bl >= 2:
                    # column sums, two blocks behind so they never head-of-line
                    # block the next SpMM on the PE queue.
                    pb2 = bl - 2
                    xo2 = x1_sb[:, pb2 * H:(pb2 + 1) * H]
                    hr2 = hrs[pb2]
                    cs_ps = psC.tile([128, 2 * CHT], F32, tag="cs")
                    for c in range(CHT):
                        nc.tensor.matmul(cs_ps[:, c:c + 1],
                                         xo2[:, c * 128:(c + 1) * 128],
                                         ones[:], start=True, stop=True)
                        nc.tensor.matmul(cs_ps[:, CHT + c:CHT + c + 1],
                                         hr2[:, c * 128:(c + 1) * 128],
                                         ones[:], start=True, stop=True)
                    nc.vector.tensor_add(cs_sb[:], cs_sb[:], cs_ps[:])

            # cs2 = cs_x1 + cs_hr ; bf16 for the matvec
            csb = pp.tile([128, 2 * CHT], BF16)
            nc.vector.tensor_copy(csb[:, 0:CHT], cs_sb[:, 0:CHT])
            nc.vector.tensor_add(csb[:, CHT:2 * CHT], cs_sb[:, 0:CHT],
                                 cs_sb[:, CHT:2 * CHT])
            # per-core partial g = cs_c @ Wc.T (unscaled; bf16 matvec, 2 passes)
            gout = pp.tile([1, H], F32)
            for h in range(2):                       # 2 x 384 output columns
                g_ps = psW.tile([1, 384], F32, tag="pw")
                for j in range(2 * CHT):
                    nc.tensor.matmul(g_ps[:], csb[:, j:j + 1],
                                     wct_sb[:, j * H + h * 384:j * H + (h + 1) * 384],
                                     start=(j == 0), stop=(j == 2 * CHT - 1))
                nc.vector.tensor_copy(gout[:, h * 384:(h + 1) * 384], g_ps[:])
            nc.sync.dma_start(gpart[:], gout[:])
    nc.compile()
    return nc


# ----------------------------------------------------------------------------
# Launch C: matmul-free finalize, channel-major [128, CHT] layout.
# row0_cm = sum_c parts_c / n + bc_cm + x0_cm
# ----------------------------------------------------------------------------

def _build_C(n):
    nc = bacc.Bacc("TRN2", target_bir_lowering=False, debug=False,
                   enable_asserts=False, num_devices=N_CORES)
    # parts_cm[p, j*N_CORES + c] = gpart_c[j*128 + p]
    parts = nc.dram_tensor("parts", [128, CHT * N_CORES], F32, kind="ExternalInput")
    bcr = nc.dram_tensor("bcr", [128, CHT], F32, kind="ExternalInput")
    x0r = nc.dram_tensor("x0r", [128, CHT], F32, kind="ExternalInput")
    row0 = nc.dram_tensor("row0", [128, CHT], F32, kind="ExternalOutput")

    with tile.TileContext(nc) as tc:
        with tc.tile_pool(name="sb", bufs=1) as sb:
            pt = sb.tile([128, CHT * N_CORES], F32)
            bc_sb = sb.tile([128, CHT], F32)
            x0_sb = sb.tile([128, CHT], F32)
            nc.sync.dma_start(pt[:], parts[:])
            nc.sync.dma_start(bc_sb[:], bcr[:])
            nc.sync.dma_start(x0_sb[:], x0r[:])
            red = sb.tile([128, CHT], F32)
            nc.vector.tensor_reduce(
                red[:], pt[:].rearrange("p (j c) -> p j c", c=N_CORES),
                mybir.AxisListType.X, ALU.add)
            out_sb = sb.tile([128, CHT], F32)
            nc.vector.tensor_scalar(out_sb[:], red[:], 1.0 / 4096.0, None,
                                    ALU.mult)
            nc.vector.tensor_add(out_sb[:], out_sb[:], bc_sb[:])
            nc.vector.tensor_add(out_sb[:], out_sb[:], x0_sb[:])
            nc.sync.dma_start(row0[:], out_sb[:])
    nc.compile()
    return nc


def _pack_wt(w, dtype=np.float32):
    """[Hout, Hin] weight -> partition-major packed W.T tiles [128, (Hin/128)*Hout]:
    out[p, c*Hout + o] = W[o, c*128 + p]"""
    h_out, h_in = w.shape
    nt = h_in // 128
    out = np.empty((128, nt * h_out), dtype=np.float32)
    for c in range(nt):
        out[:, c * h_out:(c + 1) * h_out] = w[:, c * 128:(c + 1) * 128].T
    return np.ascontiguousarray(out.astype(dtype))


def _to_cm(v):
    """[768] -> channel-major [128, 6]: out[p, j] = v[j*128+p]."""
    return np.ascontiguousarray(v.reshape(CHT, 128).T.astype(np.float32))


def kernel(**inputs):
    x = np.asarray(inputs["x"], dtype=np.float32)
    w1 = np.asarray(inputs["W1"], dtype=np.float32)
    b1 = np.asarray(inputs["b1"], dtype=np.float32)
    t1 = np.float32(np.asarray(inputs["t1"]))
    w2 = np.asarray(inputs["W2"], dtype=np.float32)
    b2 = np.asarray(inputs["b2"], dtype=np.float32)
    t2 = np.float32(np.asarray(inputs["t2"]))
    ln_g = np.asarray(inputs["ln_g"], dtype=np.float32)
    ln_b = np.asarray(inputs["ln_b"], dtype=np.float32)
    wc = np.asarray(inputs["Wc"], dtype=np.float32)
    bc = np.asarray(inputs["bc"], dtype=np.float32)
    ei = np.asarray(inputs["edge_index"])

    n = x.shape[1]
    ln_trivial = bool(np.all(ln_g == 1.0) and np.all(ln_b == 0.0))
    ekey = (ei.shape[1], n, ln_trivial,
            int(np.bitwise_xor.reduce(ei[0].astype(np.int64) * 31 + ei[1])))
    if ekey not in _cache:
        prep = _prepare(ei, n)
        progs = dict(A=_build_A(prep), B=_build_B(prep, ln_trivial),
                     C=_build_C(n))
        _cache[ekey] = (prep, progs)
    prep, progs = _cache[ekey]
    perm, bpc = prep["perm"], prep["bpc"]

    xp = np.ascontiguousarray(x[0][perm])            # permuted node features
    t1r = np.full((128, 1), t1, dtype=np.float32)
    t2r = np.full((128, 1), t2, dtype=np.float32)
    w1t = _pack_wt(w1, ml_dtypes.bfloat16)
    w2t = _pack_wt(w2, ml_dtypes.bfloat16)
    wct = _pack_wt(wc, ml_dtypes.bfloat16)
    b1r = np.ascontiguousarray(np.broadcast_to(b1, (128, H)))
    b2r = np.ascontiguousarray(np.broadcast_to(b2, (128, H)))
    lngr = np.ascontiguousarray(np.broadcast_to(ln_g, (128, H)))
    lnbr = np.ascontiguousarray(np.broadcast_to(ln_b, (128, H)))

    cores = list(range(N_CORES))

    # --- launch A: conv1 -> x1(bf16) + qw2 ---
    mapsA = [dict(xin=_win_slice(xp, prep, c), ab=prep["abands"][c],
                  wt=w1t, br=b1r, ts=t1r, ts2=t2r) for c in cores]
    resA = run_bass_kernel_spmd(progs["A"], mapsA, core_ids=cores)
    x1 = np.concatenate([resA.results[c]["x1out"] for c in cores], axis=0)
    qw2 = np.concatenate([resA.results[c]["qw2out"] for c in cores], axis=0)

    # --- launch B: conv2 + LN + colsums + partial Wc matvec ---
    mapsB = []
    for c in cores:
        m = dict(qwin=_win_slice(qw2, prep, c),
                 x1own=x1[c * bpc * 128:(c + 1) * bpc * 128],
                 ab=prep["abands"][c], wt=w2t, br=b2r, wct=wct)
        if not ln_trivial:
            m["lngr"] = lngr
            m["lnbr"] = lnbr
        mapsB.append(m)
    resB = run_bass_kernel_spmd(progs["B"], mapsB, core_ids=cores)
    g = np.stack([resB.results[c]["gpart"][0] for c in cores])   # [8, 768]
    # channel-major stack: parts_cm[p, j*8+c] = g[c, j*128+p]
    parts_cm = np.ascontiguousarray(
        g.reshape(N_CORES, CHT, 128).transpose(2, 1, 0).reshape(128, CHT * N_CORES))

    # --- launch C: finalize row0 ---
    mapsC = [dict(parts=parts_cm, bcr=_to_cm(bc), x0r=_to_cm(x[0, 0]))
             for _ in cores]
    resC = run_bass_kernel_spmd(progs["C"], mapsC, core_ids=cores)
    row0 = resC.results[0]["row0"].T.reshape(H)      # channel-major -> [768]

    out = x.copy()
    out[0, 0, :] = row0
    return out


# revision 39
# speedup vs baseline: 1.1573x; 1.1534x over previous
"""Trainium2 Bass kernel for a 2-layer GENConv (softmax aggr) + LayerNorm GNN block.

Distribution: graph-partitioned across 8 NeuronCores. Nodes are reordered by a
Fiedler-vector (spectral 1D) layout so the adjacency becomes banded; the
per-channel softmax aggregation collapses to two banded-SpMM matmuls because
GENConv softmax logits depend only on the source node:

  r = relu(x); w = exp(t*r); q = w*r
  num = A @ q;  den = A @ w;  agg = num/den        (the max-shift cancels; the
  1e-7 message eps shifts agg by exactly 1e-7 — far below tolerance — dropped)

Each core owns 4 contiguous dst blocks of 128 nodes; its banded A^T slab and
the qw window it contracts against are uniform across cores (SPMD), with
per-core variation expressed purely through input data (zero-padded bands).

Three SPMD launches (host work between them is pure data movement):
  A: conv1, software-pipelined (SpMM of block b+1 issued before the epilogue
     of block b so the PE never drains); emits x1 in bf16 and conv2's message
     tensors q2|w2 = relu(x1)*exp(t2*relu(x1)), exp(t2*relu(x1)).
  B: conv2 from the precomputed qw2 window (no window elementwise at all),
     LayerNorm via bn_stats, channel-major column sums off bf16 tiles
     (cs2 = cs_x1 + cs_relu(LN) — x2 itself is never materialized), and a
     per-core partial Wc matvec g_c = colsums_c @ Wc.T (bf16).
  C: tiny matmul-free finalize in channel-major [128,6] layout:
     row0 = sum_c g_c / n + bc + x0.
"""

import ml_dtypes
import numpy as np

import concourse.bass as bass
import concourse.bacc as bacc
import concourse.mybir as mybir
import concourse.tile as tile
import concourse.masks as masks
from concourse.bass_utils import run_bass_kernel_spmd

F32 = mybir.dt.float32
BF16 = mybir.dt.bfloat16
F8E4 = mybir.dt.float8e4
AF = mybir.ActivationFunctionType
ALU = mybir.AluOpType

N_CORES = 8
H = 768
CHT = H // 128           # channel tiles = 6
LN_EPS = 1e-5

_cache = {}


# ----------------------------------------------------------------------------
# Host-side graph preprocessing (index work only — no float math on data).
# ----------------------------------------------------------------------------

def _band_struct(ns, nd, n, bpc):
    """Per-block source-tile extents [st, en) (in 128-tiles) of the permuted
    adjacency, and the per-slot window ranges shared across cores."""
    nb = n // 128
    order = np.lexsort((ns, nd))
    ns_s, nd_s = ns[order], nd[order]
    starts = np.searchsorted(nd_s, np.arange(0, n, 128))
    ends = np.searchsorted(nd_s, np.arange(128, n + 1, 128))
    st = np.empty(nb, dtype=np.int64)
    en = np.empty(nb, dtype=np.int64)
    for b in range(nb):
        s = ns_s[starts[b]:ends[b]]
        st[b] = s.min() // 128
        en[b] = s.max() // 128 + 1
    pmax_t = int((np.arange(nb) - st).max())         # tiles left of own block
    # window origin (tile) for core c is c*bpc - pmax_t; slot ranges are the
    # max hull across cores in window-tile coordinates
    slot_lo, slot_hi = [], []
    for s in range(bpc):
        org = np.arange(N_CORES) * bpc - pmax_t
        blocks = np.arange(N_CORES) * bpc + s
        slot_lo.append(int((st[blocks] - org).min()))
        slot_hi.append(int((en[blocks] - org).max()))
    wlo = min(slot_lo)
    whi = max(slot_hi)
    slot_lo = [lo - wlo for lo in slot_lo]
    slot_hi = [hi - wlo for hi in slot_hi]
    return st, en, pmax_t, wlo, whi, slot_lo, slot_hi


def _ordering(src, dst, n, bpc):
    """1D layout minimizing the per-slot banded-SpMM tile count: scan linear
    directions in the span of Laplacian eigenvectors 1..3 (the near-degenerate
    first modes of a 3D point cloud mix arbitrarily; a pure axis mode gives
    the narrowest band). Falls back to the Fiedler vector / identity."""
    import scipy.sparse as sp
    a = sp.csr_matrix(
        (np.ones(len(src), dtype=np.float64), (dst, src)), shape=(n, n)
    )
    asym = ((a + a.T) > 0).astype(np.float64)
    try:
        from scipy.sparse.linalg import eigsh
        lap = sp.diags(np.asarray(asym.sum(1)).ravel()) - asym
        _, vecs = eigsh(lap, k=4, sigma=-1e-4, which="LM")
        emb = vecs[:, 1:4]
    except Exception:
        return np.arange(n, dtype=np.int64)

    inv = np.empty(n, dtype=np.int64)

    def cost(perm):
        inv[perm] = np.arange(n)
        _, _, _, wlo, whi, slot_lo, slot_hi = _band_struct(
            inv[src], inv[dst], n, bpc)
        return (sum(hi - lo for lo, hi in zip(slot_lo, slot_hi)), whi - wlo)

    rngs = np.random.RandomState(42)
    dirs = [np.eye(3)[i] for i in range(3)]
    dirs += [v / np.linalg.norm(v) for v in rngs.randn(240, 3)]
    best = None
    for u in dirs:
        perm = np.argsort(emb @ u).astype(np.int64)
        c = cost(perm)
        if best is None or c < best[0]:
            best = (c, perm)
    return best[1]


def _prepare(edge_index, n):
    import scipy.sparse as sp
    src = np.asarray(edge_index[0], dtype=np.int64)
    dst = np.asarray(edge_index[1], dtype=np.int64)
    nb = n // 128
    bpc = nb // N_CORES                     # blocks per core
    perm = _ordering(src, dst, n, bpc)      # new position i holds old node perm[i]
    inv = np.empty(n, dtype=np.int64)
    inv[perm] = np.arange(n)
    ns, nd = inv[src], inv[dst]             # edges in new coordinates

    st, en, pmax_t, wlo, whi, slot_lo, slot_hi = _band_struct(ns, nd, n, bpc)
    slot_S = [hi - lo for lo, hi in zip(slot_lo, slot_hi)]
    slot_off = np.concatenate([[0], np.cumsum(slot_S)]).astype(int)
    nxt = whi - wlo                          # window tiles per core
    own_off = pmax_t - wlo                   # window tile of first own block
    wx = nxt * 128

    # banded A^T slabs, packed per (slot, k) for contiguous DMA:
    # ab[c][p, (slot_off[s]+k)*128 + d] =
    #   #edges src=(win_org + (slot_lo[s]+k)*128 + p) -> dst=(blk(c,s)*128 + d)
    amat = sp.csr_matrix(
        (np.ones(len(ns), dtype=np.float64), (nd, ns)), shape=(n, n))
    abands = []
    for c in range(N_CORES):
        org = (c * bpc - pmax_t + wlo) * 128          # window row origin
        ab = np.zeros((128, slot_off[-1] * 128), dtype=np.float32)
        for s in range(bpc):
            blk = (c * bpc + s) * 128
            for k in range(slot_S[s]):
                r0 = org + (slot_lo[s] + k) * 128     # src rows of this tile
                a0, a1 = max(r0, 0), min(r0 + 128, n)
                if a0 >= a1:
                    continue
                sub = np.asarray(
                    amat[blk:blk + 128, a0:a1].todense(), dtype=np.float32)
                tilecol = (slot_off[s] + k) * 128
                ab[a0 - r0:a1 - r0, tilecol:tilecol + 128] = sub.T
        abands.append(ab.astype(ml_dtypes.float8_e4m3))

    return dict(perm=perm, inv=inv, pmax_t=pmax_t, wlo=wlo, nxt=nxt, wx=wx,
                own_off=own_off, slot_lo=slot_lo, slot_S=slot_S,
                slot_off=slot_off, bpc=bpc, abands=abands)


def _win_slice(full, prep, c):
    """Window rows of `full` for core c, zero-padded."""
    n = full.shape[0]
    bpc, pmax_t, wlo, wx = prep["bpc"], prep["pmax_t"], prep["wlo"], prep["wx"]
    lo = (c * bpc - pmax_t + wlo) * 128
    hi = lo + wx
    out = np.zeros((hi - lo, full.shape[1]), dtype=full.dtype)
    a, b = max(lo, 0), min(hi, n)
    out[a - lo:b - lo] = full[a:b]
    return out


# ----------------------------------------------------------------------------
# Shared Bass fragments.
# ----------------------------------------------------------------------------

def _spmm_block(nc, agg, ab_sb, qw, bl, prep):
    """agg[128,2H] (PSUM) += banded A^T slab tiles x qw window tiles.
    fp8 DoubleRow: each matmul contracts a PAIR of 128-row k-tiles
    (lhsT [128,2,128], rhs [128,2,512]) at 2x PE rate; odd tail single."""
    S = prep["slot_S"][bl]
    off = prep["slot_off"][bl]
    lo = prep["slot_lo"][bl]
    k = 0
    while k < S:
        pair = 2 if k + 1 < S else 1
        s = lo + k                      # first window tile of this pair
        last = k + pair >= S
        if pair == 2:
            at = ab_sb[:, (off + k) * 128:(off + k + 2) * 128].rearrange(
                "p (j d) -> p j d", d=128)
            qp = qw[:, s * 2 * H:(s + 2) * 2 * H].rearrange(
                "p (j d) -> p j d", d=2 * H)
            for ch in range(3):         # 1536 free = 3 x 512
                qv = qp[:, :, ch * 512:(ch + 1) * 512]
                nc.tensor.matmul(
                    agg[:, ch * 512:(ch + 1) * 512], at, qv,
                    start=(k == 0), stop=last,
                    perf_mode=mybir.MatmulPerfMode.DoubleRow,
                )
        else:
            at = ab_sb[:, (off + k) * 128:(off + k + 1) * 128]
            for ch in range(3):
                nc.tensor.matmul(
                    agg[:, ch * 512:(ch + 1) * 512], at,
                    qw[:, s * 2 * H + ch * 512:s * 2 * H + (ch + 1) * 512],
                    start=(k == 0), stop=last,
                )
        k += pair


def _div_res(nc, ep, agg, xo):
    """m_bf (bf16) = 16 * agg[:, :H] / agg[:, H:] + xo  (softmax divide +
    residual; the 16 undoes the q/16 fp8-range scaling of the messages).
    Runs in two half-width passes so the first transposes can start ~1.5us
    earlier (shorter critical chain on the last block)."""
    rec = ep.tile([128, H], F32, tag="rec")
    mtmp = ep.tile([128, H], F32, tag="mtmp")
    m_bf = ep.tile([128, H], BF16, tag="m_bf")
    hh = H // 2
    for i in range(2):
        sl = slice(i * hh, (i + 1) * hh)
        nc.vector.reciprocal_approx_fast(rec[:, sl], agg[:, H + i * hh:H + (i + 1) * hh])
        nc.vector.tensor_mul(mtmp[:, sl], agg[:, sl], rec[:, sl])
        nc.vector.scalar_tensor_tensor(m_bf[:, sl], mtmp[:, sl], 16.0, xo[:, sl],
                                       ALU.mult, ALU.add)
    return m_bf


def _mlp_block(nc, pW, ep, m_bf, wt_sb, ident, br_sb, xn, pT=None):
    """xn[128,H] (SBUF f32) = m_bf @ W.T + b via 6 transposes + 12 matmuls.
    pW is a single-bank PSUM pool reused for both output passes (sequential
    requests serialize safely); transposes go to pT (own bank) if given."""
    tp = (pT or pW).tile([128, H], BF16, tag="pt" if pT else "pw")
    for c in range(CHT):
        nc.tensor.transpose(tp[:, c * 128:(c + 1) * 128],
                            m_bf[:, c * 128:(c + 1) * 128], ident[:])
    mt = ep.tile([128, H], BF16, tag="mt")
    for c in range(CHT):
        nc.scalar.copy(mt[:, c * 128:(c + 1) * 128], tp[:, c * 128:(c + 1) * 128])
    xps1 = pW.tile([128, 512], F32, tag="pw")
    for c in range(CHT):
        nc.tensor.matmul(xps1[:], mt[:, c * 128:(c + 1) * 128],
                         wt_sb[:, c * H:c * H + 512],
                         start=(c == 0), stop=(c == CHT - 1))
    nc.vector.tensor_add(xn[:, 0:512], xps1[:], br_sb[:, 0:512])
    xps2 = pW.tile([128, 256], F32, tag="pw")
    for c in range(CHT):
        nc.tensor.matmul(xps2[:], mt[:, c * 128:(c + 1) * 128],
                         wt_sb[:, c * H + 512:(c + 1) * H],
                         start=(c == 0), stop=(c == CHT - 1))
    nc.vector.tensor_add(xn[:, 512:H], xps2[:], br_sb[:, 512:H])


# ----------------------------------------------------------------------------
# Launch A: conv1 + x1(bf16) + qw2 production.
# ----------------------------------------------------------------------------

def _build_A(prep):
    wx, bpc = prep["wx"], prep["bpc"]
    own_off = prep["own_off"]               # window tile index of first own block
    nxt = prep["nxt"]                       # source-window tiles
    abt = int(prep["slot_off"][-1])         # total ab tiles
    nc = bacc.Bacc("TRN2", target_bir_lowering=False, debug=False,
                   enable_asserts=False, num_devices=N_CORES)
    xin = nc.dram_tensor("xin", [wx, H], F32, kind="ExternalInput")
    ab = nc.dram_tensor("ab", [128, abt * 128], F8E4, kind="ExternalInput")
    wt = nc.dram_tensor("wt", [128, CHT * H], BF16, kind="ExternalInput")
    br = nc.dram_tensor("br", [128, H], F32, kind="ExternalInput")
    ts = nc.dram_tensor("ts", [128, 1], F32, kind="ExternalInput")
    ts2 = nc.dram_tensor("ts2", [128, 1], F32, kind="ExternalInput")
    x1out = nc.dram_tensor("x1out", [bpc * 128, H], BF16, kind="ExternalOutput")
    qw2out = nc.dram_tensor("qw2out", [bpc * 128, 2 * H], F8E4, kind="ExternalOutput")

    xin_r = xin.rearrange("(n p) d -> n p d", p=128)
    x1out_r = x1out.rearrange("(n p) d -> n p d", p=128)
    qw2out_r = qw2out.rearrange("(n p) d -> n p d", p=128)

    with tile.TileContext(nc) as tc:
        with (
            tc.tile_pool(name="persist", bufs=1) as pp,
            tc.tile_pool(name="epi", bufs=2) as ep,
            tc.tile_pool(name="psA", bufs=2, space="PSUM") as psA,
            tc.tile_pool(name="psW", bufs=1, space="PSUM") as psW,
            tc.tile_pool(name="psT", bufs=1, space="PSUM") as psT,
        ):
            xw = pp.tile([128, nxt * H], F32)            # full f32 window
            rb = pp.tile([128, nxt * H], BF16)           # r' = relu(x)/16
            qw = pp.tile([128, nxt * 2 * H], F8E4)       # [q' | w] per window tile
            ab_sb = pp.tile([128, abt * 128], F8E4)
            wt_sb = pp.tile([128, CHT * H], BF16)
            br_sb = pp.tile([128, H], F32)
            ts_sb = pp.tile([128, 1], F32)
            ts16 = pp.tile([128, 1], F32)
            ts216 = pp.tile([128, 1], F32)
            ts2_sb = pp.tile([128, 1], F32)
            ident = pp.tile([128, 128], BF16)
            nb4 = pp.tile([128, 1], F32)
            masks.make_identity(nc, ident[:])
            nc.gpsimd.memset(nb4[:], -4.0)
            nc.sync.dma_start(ts_sb[:], ts[:])
            nc.sync.dma_start(ts2_sb[:], ts2[:])
            nc.vector.tensor_scalar_mul(ts16[:], ts_sb[:], 16.0)
            nc.vector.tensor_scalar_mul(ts216[:], ts2_sb[:], 16.0)

            # window DMAs first (they gate the elementwise chain and SpMM);
            # ab interleaved per quarter slab; weights/bias later.
            abw = abt * 128
            q4 = (abw // 4) // 128 * 128
            absl = [(0, q4), (q4, 2 * q4), (2 * q4, 3 * q4), (3 * q4, abw)]
            nc.sync.dma_start(ab_sb[:, absl[0][0]:absl[0][1]],
                              ab[:, absl[0][0]:absl[0][1]])
            for s in range(nxt):
                nc.sync.dma_start(xw[:, s * H:(s + 1) * H], xin_r[s])
                if s in (2, 5, 8):
                    i = s // 3 + 1
                    nc.sync.dma_start(ab_sb[:, absl[i][0]:absl[i][1]],
                                      ab[:, absl[i][0]:absl[i][1]])
                if s == 11:
                    nc.sync.dma_start(wt_sb[:], wt[:])
                if s == 13:
                    nc.sync.dma_start(br_sb[:], br[:])

            # window pass (2 tiles per op):
            # r' = relu(x)/16 (bf16), w = exp(16t*r') (fp8), q' = r'*w (fp8)
            for s in range(0, nxt, 2):
                pair = min(2, nxt - s)
                xt = xw[:, s * H:(s + pair) * H].rearrange(
                    "p (a d) -> p a d", d=H)
                rt = rb[:, s * H:(s + pair) * H].rearrange(
                    "p (a d) -> p a d", d=H)
                qv = qw[:, 2 * s * H:2 * (s + pair) * H].rearrange(
                    "p (a d) -> p a d", d=2 * H)
                qs, ws = qv[:, :, 0:H], qv[:, :, H:2 * H]
                if s % 4 == 0:
                    nc.vector.tensor_scalar(rt, xt, 0.0, 1.0 / 16.0,
                                            ALU.max, ALU.mult)
                else:
                    nc.scalar.activation(rt, xt, AF.Relu, scale=1.0 / 16.0)
                nc.scalar.activation(ws, rt, AF.Exp, scale=ts16[:, 0:1])
                nc.vector.tensor_mul(qs, rt, ws)

            # software pipeline: SpMM(bl) issued before epilogue(bl-1)
            aggs = [None] * bpc
            for bl in range(bpc + 1):
                if bl < bpc:
                    agg = psA.tile([128, 2 * H], F32, tag="agg")
                    _spmm_block(nc, agg, ab_sb, qw, bl, prep)
                    aggs[bl] = agg
                if bl >= 1:
                    pb = bl - 1
                    xo = xw[:, (own_off + pb) * H:(own_off + pb + 1) * H]
                    m_bf = _div_res(nc, ep, aggs[pb], xo)
                    xn = ep.tile([128, H], BF16, tag="xn")
                    _mlp_block(nc, psW, ep, m_bf, wt_sb, ident, br_sb, xn,
                               pT=psT)
                    nc.sync.dma_start(x1out_r[pb], xn[:])
                    # conv2 message tensors for the own rows (fp8, scaled):
                    # r2' = relu(x1)/16, w2 = exp(16*t2*r2' - 4), q2' = r2'*w2
                    # (the e^-4 shift and /16 scale cancel in the softmax)
                    r2b = ep.tile([128, H], BF16, tag="r2b")
                    nc.scalar.activation(r2b[:], xn[:], AF.Relu, scale=1.0 / 16.0)
                    qw2 = ep.tile([128, 2 * H], F8E4, tag="qw2")
                    nc.scalar.activation(qw2[:, H:2 * H], r2b[:], AF.Exp,
                                         scale=ts216[:, 0:1], bias=nb4[:, 0:1])
                    nc.vector.tensor_mul(qw2[:, 0:H], r2b[:], qw2[:, H:2 * H])
                    nc.sync.dma_start(qw2out_r[pb], qw2[:])
    nc.compile()
    return nc


# ----------------------------------------------------------------------------
# Launch B: conv2 + LN + colsums + partial Wc matvec.
# ----------------------------------------------------------------------------

def _build_B(prep, ln_trivial):
    wx, bpc = prep["wx"], prep["bpc"]
    nxt = prep["nxt"]
    abt = int(prep["slot_off"][-1])
    nc = bacc.Bacc("TRN2", target_bir_lowering=False, debug=False,
                   enable_asserts=False, num_devices=N_CORES)
    qwin = nc.dram_tensor("qwin", [wx, 2 * H], F8E4, kind="ExternalInput")
    x1own = nc.dram_tensor("x1own", [bpc * 128, H], BF16, kind="ExternalInput")
    ab = nc.dram_tensor("ab", [128, abt * 128], F8E4, kind="ExternalInput")
    wt = nc.dram_tensor("wt", [128, CHT * H], BF16, kind="ExternalInput")
    br = nc.dram_tensor("br", [128, H], F32, kind="ExternalInput")
    wct = nc.dram_tensor("wct", [128, 2 * CHT * H], BF16, kind="ExternalInput")
    if not ln_trivial:
        lngr = nc.dram_tensor("lngr", [128, H], F32, kind="ExternalInput")
        lnbr = nc.dram_tensor("lnbr", [128, H], F32, kind="ExternalInput")
    gpart = nc.dram_tensor("gpart", [1, H], F32, kind="ExternalOutput")

    qwin_r = qwin.rearrange("(n p) d -> n p d", p=128)
    x1_r = x1own.rearrange("(n p) d -> n p d", p=128)

    with tile.TileContext(nc) as tc:
        with (
            tc.tile_pool(name="persist", bufs=1) as pp,
            tc.tile_pool(name="epi", bufs=2) as ep,
            tc.tile_pool(name="psA", bufs=2, space="PSUM") as psA,
            tc.tile_pool(name="psW", bufs=1, space="PSUM") as psW,
            tc.tile_pool(name="psT", bufs=1, space="PSUM") as psT,
        ):
            qw = pp.tile([128, nxt * 2 * H], F8E4)
            x1_sb = pp.tile([128, bpc * H], BF16)
            ab_sb = pp.tile([128, abt * 128], F8E4)
            wt_sb = pp.tile([128, CHT * H], BF16)
            br_sb = pp.tile([128, H], F32)
            wct_sb = pp.tile([128, 2 * CHT * H], BF16)
            ident = pp.tile([128, 128], BF16)
            ones = pp.tile([128, 1], BF16)
            cs_sb = pp.tile([128, 2 * CHT], F32)
            lneps = pp.tile([128, 1], F32)
            masks.make_identity(nc, ident[:])
            nc.gpsimd.memset(ones[:], 1.0)
            nc.gpsimd.memset(cs_sb[:], 0.0)
            nc.gpsimd.memset(lneps[:], LN_EPS)

            # DMA order: ab slab (gates SpMM) interleaved with window tiles;
            # x1/wt/br next; wct (needed only at the end) last.
            abw = abt * 128
            q4 = (abw // 4) // 128 * 128
            absl = [(0, q4), (q4, 2 * q4), (2 * q4, 3 * q4), (3 * q4, abw)]
            nc.sync.dma_start(ab_sb[:, absl[0][0]:absl[0][1]],
                              ab[:, absl[0][0]:absl[0][1]])
            for s in range(nxt):
                nc.sync.dma_start(qw[:, s * 2 * H:(s + 1) * 2 * H], qwin_r[s])
                if s in (2, 5, 8):
                    i = s // 3 + 1
                    nc.sync.dma_start(ab_sb[:, absl[i][0]:absl[i][1]],
                                      ab[:, absl[i][0]:absl[i][1]])
            for bl in range(bpc):
                nc.sync.dma_start(x1_sb[:, bl * H:(bl + 1) * H], x1_r[bl])
            nc.sync.dma_start(wt_sb[:], wt[:])
            nc.sync.dma_start(br_sb[:], br[:])
            if not ln_trivial:
                lng_sb = pp.tile([128, H], F32)
                lnb_sb = pp.tile([128, H], F32)
                nc.sync.dma_start(lng_sb[:], lngr[:])
                nc.sync.dma_start(lnb_sb[:], lnbr[:])
            nc.sync.dma_start(wct_sb[:], wct[:])

            aggs = [None] * bpc
            hrs = [None] * bpc
            for bl in range(bpc + 2):
                if bl < bpc:
                    agg = psA.tile([128, 2 * H], F32, tag="agg")
                    _spmm_block(nc, agg, ab_sb, qw, bl, prep)
                    aggs[bl] = agg
                if 1 <= bl <= bpc:
                    pb = bl - 1
                    xo = x1_sb[:, pb * H:(pb + 1) * H]
                    m_bf = _div_res(nc, ep, aggs[pb], xo)
                    xn = ep.tile([128, H], F32, tag="xn")
                    _mlp_block(nc, psW, ep, m_bf, wt_sb, ident, br_sb, xn,
                               pT=psT)

                    # LayerNorm stats via bn_stats (3 x 256 subgroups)
                    stats = ep.tile([128, 3, 6], F32, tag="stats")
                    xn_g = xn[:].rearrange("p (a b) -> p a b", b=256)
                    for g in range(3):
                        nc.vector.bn_stats(stats[:, g, :], xn_g[:, g, :])
                    mv = ep.tile([128, 2], F32, tag="mv")
                    nc.vector.bn_aggr(mv[:], stats[:])
                    var = ep.tile([128, 1], F32, tag="var")
                    nc.vector.tensor_scalar(var[:], mv[:, 1:2], lneps[:, 0:1],
                                            None, ALU.add)
                    rstd = ep.tile([128, 1], F32, tag="rstd")
                    nc.vector.reciprocal_approx_fast(rstd[:], var[:])
                    nc.scalar.sqrt(rstd[:], rstd[:])
                    nmr = ep.tile([128, 1], F32, tag="nmr")
                    nc.vector.tensor_scalar(nmr[:], mv[:, 0:1], rstd[:, 0:1],
                                            -1.0, ALU.mult, ALU.mult)
                    hr = ep.tile([128, H], BF16, tag="hr")
                    if ln_trivial:
                        # ln_g == 1, ln_b == 0: relu(LN(x)) in one activation
                        nc.scalar.activation(hr[:], xn[:], AF.Relu,
                                             bias=nmr[:, 0:1], scale=rstd[:, 0:1])
                    else:
                        hn = ep.tile([128, H], F32, tag="hn")
                        nc.scalar.activation(hn[:], xn[:], AF.Identity,
                                             bias=nmr[:, 0:1], scale=rstd[:, 0:1])
                        nc.vector.tensor_mul(hn[:], hn[:], lng_sb[:])
                        nc.vector.tensor_add(hn[:], hn[:], lnb_sb[:])
                        nc.scalar.activation(hr[:], hn[:], AF.Relu)
                    hrs[pb] = hr
                if bl >= 2:
                    # column sums, two blocks behind so they never head-of-line
                    # block the next SpMM on the PE queue.
                    pb2 = bl - 2
                    xo2 = x1_sb[:, pb2 * H:(pb2 + 1) * H]
                    hr2 = hrs[pb2]
                    cs_ps = psW.tile([128, 2 * CHT], F32, tag="pw")
                    for c in range(CHT):
                        nc.tensor.matmul(cs_ps[:, c:c + 1],
                                         xo2[:, c * 128:(c + 1) * 128],
                                         ones[:], start=True, stop=True)
                        nc.tensor.matmul(cs_ps[:, CHT + c:CHT + c + 1],
                                         hr2[:, c * 128:(c + 1) * 128],
                                         ones[:], start=True, stop=True)
                    nc.vector.tensor_add(cs_sb[:], cs_sb[:], cs_ps[:])

            # cs2 = cs_x1 + cs_hr ; bf16 for the matvec
            csb = pp.tile([128, 2 * CHT], BF16)
            nc.vector.tensor_copy(csb[:, 0:CHT], cs_sb[:, 0:CHT])
            nc.vector.tensor_add(csb[:, CHT:2 * CHT], cs_sb[:, 0:CHT],
                                 cs_sb[:, CHT:2 * CHT])
            # per-core partial g = cs_c @ Wc.T (unscaled; bf16 matvec, 2 passes)
            gout = pp.tile([1, H], F32)
            for h in range(2):                       # 2 x 384 output columns
                g_ps = psW.tile([1, 384], F32, tag="pw")
                for j in range(2 * CHT):
                    nc.tensor.matmul(g_ps[:], csb[:, j:j + 1],
                                     wct_sb[:, j * H + h * 384:j * H + (h + 1) * 384],
                                     start=(j == 0), stop=(j == 2 * CHT - 1))
                nc.vector.tensor_copy(gout[:, h * 384:(h + 1) * 384], g_ps[:])
            nc.sync.dma_start(gpart[:], gout[:])
    nc.compile()
    return nc


# ----------------------------------------------------------------------------
# Launch C: matmul-free finalize, channel-major [128, CHT] layout.
# row0_cm = sum_c parts_c / n + bc_cm + x0_cm
# ----------------------------------------------------------------------------

def _build_C(n):
    nc = bacc.Bacc("TRN2", target_bir_lowering=False, debug=False,
                   enable_asserts=False, num_devices=N_CORES)
    # parts_cm[p, j*N_CORES + c] = gpart_c[j*128 + p]
    parts = nc.dram_tensor("parts", [128, CHT * N_CORES], F32, kind="ExternalInput")
    bcr = nc.dram_tensor("bcr", [128, CHT], F32, kind="ExternalInput")
    x0r = nc.dram_tensor("x0r", [128, CHT], F32, kind="ExternalInput")
    row0 = nc.dram_tensor("row0", [128, CHT], F32, kind="ExternalOutput")

    with tile.TileContext(nc) as tc:
        with tc.tile_pool(name="sb", bufs=1) as sb:
            pt = sb.tile([128, CHT * N_CORES], F32)
            bc_sb = sb.tile([128, CHT], F32)
            x0_sb = sb.tile([128, CHT], F32)
            nc.sync.dma_start(pt[:], parts[:])
            nc.sync.dma_start(bc_sb[:], bcr[:])
            nc.sync.dma_start(x0_sb[:], x0r[:])
            red = sb.tile([128, CHT], F32)
            nc.vector.tensor_reduce(
                red[:], pt[:].rearrange("p (j c) -> p j c", c=N_CORES),
                mybir.AxisListType.X, ALU.add)
            out_sb = sb.tile([128, CHT], F32)
            nc.vector.tensor_scalar(out_sb[:], red[:], 1.0 / 4096.0, None,
                                    ALU.mult)
            nc.vector.tensor_add(out_sb[:], out_sb[:], bc_sb[:])
            nc.vector.tensor_add(out_sb[:], out_sb[:], x0_sb[:])
            nc.sync.dma_start(row0[:], out_sb[:])
    nc.compile()
    return nc


def _pack_wt(w, dtype=np.float32):
    """[Hout, Hin] weight -> partition-major packed W.T tiles [128, (Hin/128)*Hout]:
    out[p, c*Hout + o] = W[o, c*128 + p]"""
    h_out, h_in = w.shape
    nt = h_in // 128
    out = np.empty((128, nt * h_out), dtype=np.float32)
    for c in range(nt):
        out[:, c * h_out:(c + 1) * h_out] = w[:, c * 128:(c + 1) * 128].T
    return np.ascontiguousarray(out.astype(dtype))


def _to_cm(v):
    """[768] -> channel-major [128, 6]: out[p, j] = v[j*128+p]."""
    return np.ascontiguousarray(v.reshape(CHT, 128).T.astype(np.float32))


def kernel(**inputs):
    x = np.asarray(inputs["x"], dtype=np.float32)
    w1 = np.asarray(inputs["W1"], dtype=np.float32)
    b1 = np.asarray(inputs["b1"], dtype=np.float32)
    t1 = np.float32(np.asarray(inputs["t1"]))
    w2 = np.asarray(inputs["W2"], dtype=np.float32)
    b2 = np.asarray(inputs["b2"], dtype=np.float32)
    t2 = np.float32(np.asarray(inputs["t2"]))
    ln_g = np.asarray(inputs["ln_g"], dtype=np.float32)
    ln_b = np.asarray(inputs["ln_b"], dtype=np.float32)
    wc = np.asarray(inputs["Wc"], dtype=np.float32)
    bc = np.asarray(inputs["bc"], dtype=np.float32)
    ei = np.asarray(inputs["edge_index"])

    n = x.shape[1]
    ln_trivial = bool(np.all(ln_g == 1.0) and np.all(ln_b == 0.0))
    ekey = (ei.shape[1], n, ln_trivial,
            int(np.bitwise_xor.reduce(ei[0].astype(np.int64) * 31 + ei[1])))
    if ekey not in _cache:
        prep = _prepare(ei, n)
        progs = dict(A=_build_A(prep), B=_build_B(prep, ln_trivial),
                     C=_build_C(n))
        _cache[ekey] = (prep, progs)
    prep, progs = _cache[ekey]
    perm, bpc = prep["perm"], prep["bpc"]

    xp = np.ascontiguousarray(x[0][perm])            # permuted node features
    t1r = np.full((128, 1), t1, dtype=np.float32)
    t2r = np.full((128, 1), t2, dtype=np.float32)
    w1t = _pack_wt(w1, ml_dtypes.bfloat16)
    w2t = _pack_wt(w2, ml_dtypes.bfloat16)
    wct = _pack_wt(wc, ml_dtypes.bfloat16)
    b1r = np.ascontiguousarray(np.broadcast_to(b1, (128, H)))
    b2r = np.ascontiguousarray(np.broadcast_to(b2, (128, H)))
    lngr = np.ascontiguousarray(np.broadcast_to(ln_g, (128, H)))
    lnbr = np.ascontiguousarray(np.broadcast_to(ln_b, (128, H)))

    cores = list(range(N_CORES))

    # --- launch A: conv1 -> x1(bf16) + qw2 ---
    mapsA = [dict(xin=_win_slice(xp, prep, c), ab=prep["abands"][c],
                  wt=w1t, br=b1r, ts=t1r, ts2=t2r) for c in cores]
    resA = run_bass_kernel_spmd(progs["A"], mapsA, core_ids=cores)
    x1 = np.concatenate([resA.results[c]["x1out"] for c in cores], axis=0)
    qw2 = np.concatenate([resA.results[c]["qw2out"] for c in cores], axis=0)

    # --- launch B: conv2 + LN + colsums + partial Wc matvec ---
    mapsB = []
    for c in cores:
        m = dict(qwin=_win_slice(qw2, prep, c),
                 x1own=x1[c * bpc * 128:(c + 1) * bpc * 128],
                 ab=prep["abands"][c], wt=w2t, br=b2r, wct=wct)
        if not ln_trivial:
            m["lngr"] = lngr
            m["lnbr"] = lnbr
        mapsB.append(m)
    resB = run_bass_kernel_spmd(progs["B"], mapsB, core_ids=cores)
    g = np.stack([resB.results[c]["gpart"][0] for c in cores])   # [8, 768]
    # channel-major stack: parts_cm[p, j*8+c] = g[c, j*128+p]
    parts_cm = np.ascontiguousarray(
        g.reshape(N_CORES, CHT, 128).transpose(2, 1, 0).reshape(128, CHT * N_CORES))

    # --- launch C: finalize row0 ---
    mapsC = [dict(parts=parts_cm, bcr=_to_cm(bc), x0r=_to_cm(x[0, 0]))
             for _ in cores]
    resC = run_bass_kernel_spmd(progs["C"], mapsC, core_ids=cores)
    row0 = resC.results[0]["row0"].T.reshape(H)      # channel-major -> [768]

    out = x.copy()
    out[0, 0, :] = row0
    return out
